# revision 16
# baseline (speedup 1.0000x reference)
"""CrossVerseAttention Trainium2 kernel.

Sharding: 8 cores = 2 batches x 4 head-groups. Core c handles batch c//4 and
heads [4*(c%4), 4*(c%4)+4). Attention scores are kept transposed [k, q] so the
cross-verse mask (<=200 distinct rows, verse values in [0,200)) can be added
per k-row tile via an indirect-DMA row gather; softmax runs without the max
subtraction (scores are O(1) here) with row sums produced by a ones column
appended to V inside the attn@V matmul. Partial attention outputs are
ReduceScattered (s-blocked, pre-transposed) across each 4-core batch group,
after which every core runs Wp + residual + LayerNorm on its own 512-row
slice.
"""
import os
import sys

sys.path.insert(0, "/opt/trn_rl_repo")

import numpy as np

DEBUG = bool(os.environ.get("CVK_DEBUG"))

import concourse.bacc as bacc
import concourse.bass as bass
import concourse.tile as tile
from concourse import mybir
from concourse.bass import AP
from concourse.bass_utils import run_bass_kernel_spmd
from concourse.masks import make_identity

B, S, D, H = 2, 2048, 1024, 16
DH = D // H            # 64
HPC = H // 4           # 4 heads per core
DC = HPC * DH          # 256 head dims per core
SS = S // 4            # 512 output rows per core
EPS = 1e-5
SCALE = 1.0 / float(np.sqrt(DH))
F32 = mybir.dt.float32
NQC = 4                # q chunks of 512
QC = S // NQC          # 512
NKT = S // 128         # 16 k tiles
NST = S // 128         # 16 s tiles
NDCH = D // 128        # 8 contraction chunks
GROUPS = [[0, 1, 2, 3], [4, 5, 6, 7]]

_CACHE = {}


def _stage1_mask_rows(nc, tc, vq, mdist):
    """Compute the <=256 distinct mask rows and store them to DRAM."""
    def bcast(ap, p=128):
        return AP(tensor=ap.tensor, offset=ap.offset,
                  ap=[[0, p]] + list(ap.ap))

    with (
        tc.tile_pool(name="m1", bufs=2) as m1,
        tc.tile_pool(name="m1c", bufs=1) as m1c,
        tc.tile_pool(name="m1s", bufs=2) as m1s,
    ):
        vq_b = m1c.tile([128, S], F32)
        nc.sync.dma_start(out=vq_b, in_=bcast(vq[:]))
        zq = m1c.tile([128, S], mybir.dt.int32)
        nc.vector.tensor_scalar(out=zq, in0=vq_b, scalar1=0.0, scalar2=None,
                                op0=mybir.AluOpType.is_equal)
        ones_t = m1c.tile([128, S], F32)
        nc.vector.memset(ones_t, 1.0)
        for vi in range(2):
            vcol_i = m1s.tile([128, 1], mybir.dt.int32)
            nc.gpsimd.iota(vcol_i, pattern=[[0, 1]], base=vi * 128,
                           channel_multiplier=1)
            vcol = m1s.tile([128, 1], F32)
            nc.vector.tensor_copy(out=vcol, in_=vcol_i)
            d = m1.tile([128, S], F32, tag="d")
            nc.vector.tensor_scalar_sub(out=d, in0=vq_b, scalar1=vcol)
            adiff = m1.tile([128, S], F32, tag="adiff")
            nc.vector.tensor_scalar_mul(out=adiff, in0=d, scalar1=-1.0)
            nc.vector.tensor_tensor(out=adiff, in0=d, in1=adiff,
                                    op=mybir.AluOpType.max)
            m = m1.tile([128, S], F32, tag="m")
            nc.vector.tensor_scalar_max(out=m, in0=adiff, scalar1=1.0)
            nc.vector.tensor_scalar_mul(out=m, in0=m, scalar1=10.0 / 3.0)
            nc.vector.reciprocal(out=m, in_=m)
            c1 = m1.tile([128, S], F32, tag="c1")
            nc.vector.tensor_scalar(
                out=c1, in0=adiff, scalar1=1.0, scalar2=None,
                op0=mybir.AluOpType.is_equal)
            nc.vector.scalar_tensor_tensor(
                out=m, in0=c1, scalar=0.4, in1=m,
                op0=mybir.AluOpType.mult, op1=mybir.AluOpType.add)
            nc.vector.tensor_scalar(
                out=c1, in0=adiff, scalar1=0.0, scalar2=None,
                op0=mybir.AluOpType.is_equal)
            nc.vector.scalar_tensor_tensor(
                out=m, in0=c1, scalar=0.7, in1=m,
                op0=mybir.AluOpType.mult, op1=mybir.AluOpType.add)
            nc.vector.copy_predicated(out=m, mask=zq, data=ones_t)
            if vi == 0:
                nc.vector.memset(m[0:1, :], 1.0)
            for j in range(NQC):
                nc.sync.dma_start(
                    out=mdist[j][vi * 128:(vi + 1) * 128, :],
                    in_=m[:, j * QC:(j + 1) * QC])


def _build():
    nc = bacc.Bacc("TRN2", target_bir_lowering=False, debug=False,
                   num_devices=8)

    # ---- I/O ----
    xT = nc.dram_tensor("xT", [D, S], F32, kind="ExternalInput")
    x_sl = nc.dram_tensor("x_sl", [SS, D], F32, kind="ExternalInput")
    wqT = nc.dram_tensor("wqT", [D, DC], F32, kind="ExternalInput")
    wkT = nc.dram_tensor("wkT", [D, DC], F32, kind="ExternalInput")
    wvT = nc.dram_tensor("wvT", [D, DC], F32, kind="ExternalInput")
    woT = nc.dram_tensor("woT", [DC, D], F32, kind="ExternalInput")
    wpT = nc.dram_tensor("wpT", [D, D], F32, kind="ExternalInput")
    bq = nc.dram_tensor("bq", [128, 2], F32, kind="ExternalInput")
    bk = nc.dram_tensor("bk", [128, 2], F32, kind="ExternalInput")
    bv = nc.dram_tensor("bv", [DC], F32, kind="ExternalInput")
    bo4 = nc.dram_tensor("bo4", [128, NDCH], F32, kind="ExternalInput")
    bp = nc.dram_tensor("bp", [D], F32, kind="ExternalInput")
    gamma = nc.dram_tensor("gamma", [D], F32, kind="ExternalInput")
    beta = nc.dram_tensor("beta", [D], F32, kind="ExternalInput")
    vq = nc.dram_tensor("vq", [S], F32, kind="ExternalInput")
    vk_idx = nc.dram_tensor("vk_idx", [128, NKT], mybir.dt.int32,
                            kind="ExternalInput")
    out = nc.dram_tensor("out", [SS, D], F32, kind="ExternalOutput")
    dbg = {}
    if DEBUG:
        dbg["qT"] = nc.dram_tensor("dbg_qT", [128, 2, S], F32,
                                   kind="ExternalOutput")
        dbg["kT"] = nc.dram_tensor("dbg_kT", [128, 2, S], F32,
                                   kind="ExternalOutput")
        dbg["v"] = nc.dram_tensor("dbg_v", [128, NST, HPC, 65], F32,
                                  kind="ExternalOutput")
        dbg["mt"] = nc.dram_tensor("dbg_mt", [128, QC], F32,
                                   kind="ExternalOutput")
        dbg["et"] = nc.dram_tensor("dbg_et", [128, QC], F32,
                                   kind="ExternalOutput")
        dbg["ctx"] = nc.dram_tensor("dbg_ctx", [128, NST, DC], F32,
                                    kind="ExternalOutput")
        dbg["part"] = nc.dram_tensor("dbg_part", [D, SS], F32,
                                     kind="ExternalOutput")
        dbg["rs"] = nc.dram_tensor("dbg_rs", [D, SS], F32,
                                   kind="ExternalOutput")
        dbg["opre"] = nc.dram_tensor("dbg_opre", [128, D], F32,
                                     kind="ExternalOutput")

    # ---- DRAM scratch ----
    mdist = [nc.dram_tensor(f"mdist{j}", [256, QC], F32) for j in range(NQC)]
    part_blk = nc.dram_tensor("part_blk", [4, D, SS], F32)
    rs_out = nc.dram_tensor("rs_out", [D, SS], F32)

    def bcast(ap, p=128):
        return AP(tensor=ap.tensor, offset=ap.offset,
                  ap=[[0, p]] + list(ap.ap))

    with tile.TileContext(nc) as tc:
        with tc.tile_pool(name="const", bufs=1) as const:
            # ---- small constants ----
            vki = const.tile([128, NKT], mybir.dt.int32)
            nc.sync.dma_start(out=vki, in_=vk_idx[:])
            bq_sb = const.tile([128, 2], F32)
            nc.sync.dma_start(out=bq_sb, in_=bq[:])
            bk_sb = const.tile([128, 2], F32)
            nc.sync.dma_start(out=bk_sb, in_=bk[:])
            bv_b = const.tile([128, DC], F32)
            nc.sync.dma_start(out=bv_b, in_=bcast(bv[:]))
            bo4_sb = const.tile([128, NDCH], F32)
            nc.sync.dma_start(out=bo4_sb, in_=bo4[:])
            eps_t = const.tile([128, 1], F32)
            nc.vector.memset(eps_t, EPS)
            ident = const.tile([128, 128], F32)
            make_identity(nc, ident)

            _stage1_mask_rows(nc, tc, vq, mdist)

            with tc.tile_pool(name="pctx", bufs=1) as pctx:
                ctx_sb = pctx.tile([128, NST, DC], F32)

                with tc.tile_pool(name="pqkv", bufs=1) as pqkv:
                    qT_sb = pqkv.tile([128, 2, S], F32)
                    kT_sb = pqkv.tile([128, 2, S], F32)
                    v_sb = pqkv.tile([128, NST, HPC, 65], F32)
                    nc.vector.memset(v_sb[:, :, :, 64:65], 1.0)

                    # ---- stage 2: projections qT, kT, v ----
                    with (
                        tc.tile_pool(name="wq", bufs=1) as wq_p,
                        tc.tile_pool(name="xt", bufs=8) as xt_p,
                        tc.tile_pool(name="pj", bufs=2, space="PSUM") as pj_p,
                    ):
                        wq_sb = wq_p.tile([128, NDCH, DC], F32)
                        wk_sb = wq_p.tile([128, NDCH, DC], F32)
                        wv_sb = wq_p.tile([128, NDCH, DC], F32)
                        nc.sync.dma_start(
                            out=wq_sb,
                            in_=wqT.ap().rearrange("(c p) o -> p c o", p=128))
                        nc.sync.dma_start(
                            out=wk_sb,
                            in_=wkT.ap().rearrange("(c p) o -> p c o", p=128))
                        nc.sync.dma_start(
                            out=wv_sb,
                            in_=wvT.ap().rearrange("(c p) o -> p c o", p=128))
                        xt_tiles = []
                        for ch in range(NDCH):
                            xt = xt_p.tile([128, S], F32, tag="xt",
                                           name=f"xt{ch}")
                            nc.sync.dma_start(
                                out=xt, in_=xT[ch * 128:(ch + 1) * 128, :])
                            xt_tiles.append(xt)
                        for j in range(2):
                            for sc in range(NQC):
                                ps_q = pj_p.tile([128, QC], F32, tag="psq")
                                ps_k = pj_p.tile([128, QC], F32, tag="psk")
                                for ch in range(NDCH):
                                    nc.tensor.matmul(
                                        ps_q,
                                        wq_sb[:, ch, j * 128:(j + 1) * 128],
                                        xt_tiles[ch][:, sc * QC:(sc + 1) * QC],
                                        start=(ch == 0), stop=(ch == NDCH - 1))
                                for ch in range(NDCH):
                                    nc.tensor.matmul(
                                        ps_k,
                                        wk_sb[:, ch, j * 128:(j + 1) * 128],
                                        xt_tiles[ch][:, sc * QC:(sc + 1) * QC],
                                        start=(ch == 0), stop=(ch == NDCH - 1))
                                nc.vector.tensor_scalar(
                                    out=qT_sb[:, j, sc * QC:(sc + 1) * QC],
                                    in0=ps_q, scalar1=bq_sb[:, j:j + 1],
                                    scalar2=SCALE,
                                    op0=mybir.AluOpType.add,
                                    op1=mybir.AluOpType.mult)
                                nc.vector.tensor_scalar(
                                    out=kT_sb[:, j, sc * QC:(sc + 1) * QC],
                                    in0=ps_k, scalar1=bk_sb[:, j:j + 1],
                                    scalar2=None, op0=mybir.AluOpType.add)
                        for st in range(NST):
                            ps_v = pj_p.tile([128, DC], F32, tag="psv")
                            for ch in range(NDCH):
                                nc.tensor.matmul(
                                    ps_v,
                                    xt_tiles[ch][:, st * 128:(st + 1) * 128],
                                    wv_sb[:, ch, :],
                                    start=(ch == 0), stop=(ch == NDCH - 1))
                            nc.vector.tensor_tensor(
                                out=v_sb[:, st, :, 0:64],
                                in0=ps_v.rearrange("p (h d) -> p h d", h=HPC),
                                in1=bv_b.rearrange("p (h d) -> p h d", h=HPC),
                                op=mybir.AluOpType.add)

                    if DEBUG:
                        nc.sync.dma_start(out=dbg["qT"][:], in_=qT_sb)
                        nc.sync.dma_start(out=dbg["kT"][:], in_=kT_sb)
                        nc.sync.dma_start(out=dbg["v"][:], in_=v_sb)

                    # ---- stage 3: attention ----
                    with (
                        tc.tile_pool(name="msk", bufs=NKT + 2) as msk_p,
                        tc.tile_pool(name="sp", bufs=3) as sp_p,
                        tc.tile_pool(name="et", bufs=4) as et_p,
                        tc.tile_pool(name="rc", bufs=4) as rc_p,
                        tc.tile_pool(name="pss", bufs=4, space="PSUM") as pss_p,
                        tc.tile_pool(name="psc", bufs=2, space="PSUM") as psc_p,
                    ):
                        for qc in range(NQC):
                            mtiles = []
                            for kt in range(NKT):
                                mt = msk_p.tile([128, QC], F32, tag="mt",
                                                name=f"mt{qc}_{kt}")
                                nc.gpsimd.indirect_dma_start(
                                    out=mt, out_offset=None, in_=mdist[qc][:],
                                    in_offset=bass.IndirectOffsetOnAxis(
                                        ap=vki[:, kt:kt + 1], axis=0))
                                mtiles.append(mt)
                            for pr in range(2):
                                ctx_ps = [
                                    psc_p.tile([128, 512], F32, tag="ctx",
                                               name=f"ctx_{qc}_{pr}_{i}")
                                    for i in range(2)]
                                for kt in range(NKT):
                                    ets = []
                                    for hh in range(2):
                                        lo = hh * 64
                                        ps_s = pss_p.tile([128, QC], F32,
                                                          tag="ps_s")
                                        nc.tensor.matmul(
                                            ps_s,
                                            kT_sb[lo:lo + 64, pr,
                                                  kt * 128:(kt + 1) * 128],
                                            qT_sb[lo:lo + 64, pr,
                                                  qc * QC:(qc + 1) * QC],
                                            start=True, stop=True)
                                        sp = sp_p.tile([128, QC], F32,
                                                       tag="sp")
                                        nc.vector.tensor_add(
                                            out=sp, in0=ps_s, in1=mtiles[kt])
                                        et = et_p.tile([128, QC], F32,
                                                       tag="et")
                                        nc.scalar.activation(
                                            out=et, in_=sp,
                                            func=mybir.ActivationFunctionType.Exp)
                                        ets.append(et)
                                        if (DEBUG and qc == 0 and pr == 0
                                                and kt == 0 and hh == 0):
                                            nc.sync.dma_start(
                                                out=dbg["mt"][:],
                                                in_=mtiles[0])
                                            nc.sync.dma_start(
                                                out=dbg["et"][:], in_=et)
                                    for hh in range(2):
                                        h = 2 * pr + hh
                                        for qs in range(4):
                                            # start only on the first write to
                                            # the bank: start=True clears the
                                            # WHOLE bank's has_written bits,
                                            # so per-region starts would wipe
                                            # sibling regions' partials.
                                            nc.tensor.matmul(
                                                ctx_ps[hh][:, qs * 128:
                                                           qs * 128 + 65],
                                                ets[hh][:, qs * 128:
                                                        (qs + 1) * 128],
                                                v_sb[:, kt, h, :],
                                                start=(kt == 0 and qs == 0),
                                                stop=(kt == NKT - 1
                                                      and qs == 3),
                                                skip_group_check=True)
                                for hh in range(2):
                                    h = 2 * pr + hh
                                    cps = ctx_ps[hh].rearrange(
                                        "p (q e) -> p q e", q=4)  # e=128
                                    rec = rc_p.tile([128, 4], F32, tag="rec")
                                    nc.vector.reciprocal(out=rec,
                                                         in_=cps[:, :, 64])
                                    for qs in range(4):
                                        st = qc * 4 + qs
                                        nc.vector.tensor_scalar(
                                            out=ctx_sb[:, st,
                                                       h * 64:(h + 1) * 64],
                                            in0=cps[:, qs, 0:64],
                                            scalar1=rec[:, qs:qs + 1],
                                            scalar2=None,
                                            op0=mybir.AluOpType.mult)

                if DEBUG:
                    nc.sync.dma_start(out=dbg["ctx"][:], in_=ctx_sb)

                # ---- stage 4: transpose ctx, Wo partial, part_blk ----
                with (
                    tc.tile_pool(name="pctxT", bufs=1) as pctxT,
                    tc.tile_pool(name="wo", bufs=1) as wo_p,
                    tc.tile_pool(name="ao", bufs=3) as ao_p,
                    tc.tile_pool(name="pst", bufs=4, space="PSUM") as pst_p,
                ):
                    ctxT_sb = pctxT.tile([128, 2, S], F32)
                    for st in range(NST):
                        for pr in range(2):
                            ps_t = pst_p.tile([128, 128], F32, tag="ps_t")
                            nc.tensor.transpose(
                                ps_t, ctx_sb[:, st, pr * 128:(pr + 1) * 128],
                                ident)
                            nc.vector.tensor_copy(
                                out=ctxT_sb[:, pr, st * 128:(st + 1) * 128],
                                in_=ps_t)
                    wo_sb = wo_p.tile([128, 2, D], F32)
                    nc.sync.dma_start(
                        out=wo_sb,
                        in_=woT.ap().rearrange("(c p) o -> p c o", p=128))
                    for ot in range(NDCH):
                        for sc in range(NQC):
                            ps_a = pst_p.tile([128, QC], F32, tag="ps_a")
                            for pr in range(2):
                                nc.tensor.matmul(
                                    ps_a,
                                    wo_sb[:, pr, ot * 128:(ot + 1) * 128],
                                    ctxT_sb[:, pr, sc * QC:(sc + 1) * QC],
                                    start=(pr == 0), stop=(pr == 1))
                            ao = ao_p.tile([128, QC], F32, tag="ao")
                            nc.vector.tensor_scalar(
                                out=ao, in0=ps_a,
                                scalar1=bo4_sb[:, ot:ot + 1], scalar2=None,
                                op0=mybir.AluOpType.add)
                            nc.sync.dma_start(
                                out=part_blk[sc, ot * 128:(ot + 1) * 128, :],
                                in_=ao)

            # ---- stage 5: ReduceScatter ----
            if DEBUG:
                nc.sync.dma_start(out=dbg["part"][:], in_=part_blk[0])
            nc.gpsimd.collective_compute(
                "ReduceScatter", mybir.AluOpType.add, replica_groups=GROUPS,
                ins=[part_blk[:]], outs=[rs_out[:]])
            if DEBUG:
                nc.sync.dma_start(out=dbg["rs"][:], in_=rs_out[:])

            # ---- stage 6: Wp + residual + LayerNorm ----
            with (
                tc.tile_pool(name="wp", bufs=1) as wp_p,
                tc.tile_pool(name="rsx", bufs=1) as rsx_p,
                tc.tile_pool(name="o6", bufs=3) as o6_p,
                tc.tile_pool(name="st6", bufs=4) as st6_p,
                tc.tile_pool(name="ps6", bufs=4, space="PSUM") as ps6_p,
            ):
                wp_sb = wp_p.tile([128, NDCH, D], F32)
                nc.sync.dma_start(
                    out=wp_sb, in_=wpT.ap().rearrange("(c p) o -> p c o",
                                                      p=128))
                gamma_b = wp_p.tile([128, D], F32)
                nc.sync.dma_start(out=gamma_b, in_=bcast(gamma[:]))
                beta_b = wp_p.tile([128, D], F32)
                nc.sync.dma_start(out=beta_b, in_=bcast(beta[:]))
                bp_b = wp_p.tile([128, D], F32)
                nc.sync.dma_start(out=bp_b, in_=bcast(bp[:]))
                rsT_sb = rsx_p.tile([128, NDCH, SS], F32)
                nc.sync.dma_start(
                    out=rsT_sb,
                    in_=rs_out.ap().rearrange("(c p) s -> p c s", p=128))
                xpb = rsx_p.tile([128, 4, D], F32)
                nc.sync.dma_start(
                    out=xpb, in_=x_sl.ap().rearrange("(t p) o -> p t o",
                                                     p=128))
                for stl in range(4):
                    nc.vector.tensor_add(out=xpb[:, stl, :],
                                         in0=xpb[:, stl, :], in1=bp_b)
                for stl in range(4):
                    o_sb = o6_p.tile([128, D], F32, tag="o")
                    for nh in range(2):
                        ps_o = ps6_p.tile([128, 512], F32, tag="ps_o")
                        for ch in range(NDCH):
                            nc.tensor.matmul(
                                ps_o,
                                rsT_sb[:, ch, stl * 128:(stl + 1) * 128],
                                wp_sb[:, ch, nh * 512:(nh + 1) * 512],
                                start=(ch == 0), stop=(ch == NDCH - 1))
                        nc.vector.tensor_tensor(
                            out=o_sb[:, nh * 512:(nh + 1) * 512], in0=ps_o,
                            in1=xpb[:, stl, nh * 512:(nh + 1) * 512],
                            op=mybir.AluOpType.add)
                    if DEBUG and stl == 0:
                        nc.sync.dma_start(out=dbg["opre"][:], in_=o_sb)
                    stats = st6_p.tile([128, 2, 6], F32, tag="stats")
                    for i in range(2):
                        nc.vector.bn_stats(
                            out=stats[:, i, :],
                            in_=o_sb[:, i * 512:(i + 1) * 512])
                    mv = st6_p.tile([128, 2], F32, tag="mv")
                    nc.vector.bn_aggr(out=mv, in_=stats)
                    sd = st6_p.tile([128, 1], F32, tag="sd")
                    nc.scalar.activation(
                        out=sd, in_=mv[:, 1:2],
                        func=mybir.ActivationFunctionType.Sqrt,
                        bias=eps_t, scale=1.0)
                    rstd = st6_p.tile([128, 1], F32, tag="rstd")
                    nc.vector.reciprocal(out=rstd, in_=sd)
                    nc.vector.tensor_scalar(
                        out=o_sb, in0=o_sb, scalar1=mv[:, 0:1], scalar2=rstd,
                        op0=mybir.AluOpType.subtract, op1=mybir.AluOpType.mult)
                    nc.vector.tensor_mul(out=o_sb, in0=o_sb, in1=gamma_b)
                    nc.vector.tensor_add(out=o_sb, in0=o_sb, in1=beta_b)
                    nc.sync.dma_start(
                        out=out[stl * 128:(stl + 1) * 128, :], in_=o_sb)

    nc.compile()
    return nc


def _prep_inputs(x, verse_positions, W_q, b_q, W_k, b_k, W_v, b_v,
                 W_o, b_o, W_p, b_p, gamma, beta):
    f = np.float32
    x = np.asarray(x, f)
    vp = np.asarray(verse_positions)
    in_maps = []
    wpT = np.ascontiguousarray(np.asarray(W_p, f).T)
    for c in range(8):
        b, r = divmod(c, 4)
        sl = slice(DC * r, DC * (r + 1))
        vpb = vp[b].astype(np.int32)
        in_maps.append({
            "xT": np.ascontiguousarray(x[b].T),
            "x_sl": np.ascontiguousarray(x[b, SS * r:SS * (r + 1), :]),
            "wqT": np.ascontiguousarray(np.asarray(W_q, f)[sl, :].T),
            "wkT": np.ascontiguousarray(np.asarray(W_k, f)[sl, :].T),
            "wvT": np.ascontiguousarray(np.asarray(W_v, f)[sl, :].T),
            "woT": np.ascontiguousarray(np.asarray(W_o, f)[:, sl].T),
            "wpT": wpT,
            "bq": np.asarray(b_q, f)[sl].reshape(2, 128).T.copy(),
            "bk": np.asarray(b_k, f)[sl].reshape(2, 128).T.copy(),
            "bv": np.asarray(b_v, f)[sl].copy(),
            "bo4": (np.asarray(b_o, f) / 4.0).reshape(NDCH, 128).T.copy(),
            "bp": np.asarray(b_p, f).copy(),
            "gamma": np.asarray(gamma, f).copy(),
            "beta": np.asarray(beta, f).copy(),
            "vq": vpb.astype(f),
            "vk_idx": vpb.reshape(NKT, 128).T.copy(),
        })
    return in_maps


def kernel(**inputs):
    if "nc" not in _CACHE:
        _CACHE["nc"] = _build()
    nc = _CACHE["nc"]
    in_maps = _prep_inputs(**inputs)
    res = run_bass_kernel_spmd(nc, in_maps, core_ids=list(range(8)))
    _CACHE["last_res"] = res
    out = np.empty((B, S, D), np.float32)
    for c in range(8):
        b, r = divmod(c, 4)
        out[b, SS * r:SS * (r + 1), :] = res.results[c]["out"]
    return out


# revision 20
# speedup vs baseline: 1.4939x; 1.4939x over previous
"""CrossVerseAttention Trainium2 kernel.

Sharding: 8 cores = 2 batches x 4 head-groups. Core c handles batch c//4 and
heads [4*(c%4), 4*(c%4)+4). Attention scores are kept transposed [k, q] so the
cross-verse mask (<=200 distinct rows, verse values in [0,200)) can be added
per k-row tile via an indirect-DMA row gather; softmax runs without the max
subtraction (scores are O(1) here) with row sums produced by a ones column
appended to V inside the attn@V matmul. Partial attention outputs are
ReduceScattered (s-blocked, pre-transposed) across each 4-core batch group,
after which every core runs Wp + residual + LayerNorm on its own 512-row
slice.
"""
import os
import sys

sys.path.insert(0, "/opt/trn_rl_repo")

import numpy as np

DEBUG = bool(os.environ.get("CVK_DEBUG"))

import concourse.bacc as bacc
import concourse.bass as bass
import concourse.tile as tile
from concourse import mybir
from concourse.bass import AP
from concourse.bass_utils import run_bass_kernel_spmd
from concourse.masks import make_identity

B, S, D, H = 2, 2048, 1024, 16
DH = D // H            # 64
HPC = H // 4           # 4 heads per core
DC = HPC * DH          # 256 head dims per core
SS = S // 4            # 512 output rows per core
EPS = 1e-5
SCALE = 1.0 / float(np.sqrt(DH))
F32 = mybir.dt.float32
BF16 = mybir.dt.bfloat16
NQC = 4                # q chunks of 512
QC = S // NQC          # 512
NKT = S // 128         # 16 k tiles
NST = S // 128         # 16 s tiles
NDCH = D // 128        # 8 contraction chunks
GROUPS = [[0, 1, 2, 3], [4, 5, 6, 7]]

_CACHE = {}


def _stage1_mask_rows(nc, tc, vq, mdist):
    """Compute the <=256 distinct mask rows and store them to DRAM."""
    def bcast(ap, p=128):
        return AP(tensor=ap.tensor, offset=ap.offset,
                  ap=[[0, p]] + list(ap.ap))

    with (
        tc.tile_pool(name="m1", bufs=2) as m1,
        tc.tile_pool(name="m1c", bufs=1) as m1c,
        tc.tile_pool(name="m1s", bufs=2) as m1s,
    ):
        vq_b = m1c.tile([128, S], F32)
        nc.sync.dma_start(out=vq_b, in_=bcast(vq[:]))
        zq = m1c.tile([128, S], mybir.dt.int32)
        nc.vector.tensor_scalar(out=zq, in0=vq_b, scalar1=0.0, scalar2=None,
                                op0=mybir.AluOpType.is_equal)
        ones_t = m1c.tile([128, S], F32)
        nc.vector.memset(ones_t, 1.0)
        for vi in range(2):
            vcol_i = m1s.tile([128, 1], mybir.dt.int32)
            nc.gpsimd.iota(vcol_i, pattern=[[0, 1]], base=vi * 128,
                           channel_multiplier=1)
            vcol = m1s.tile([128, 1], F32)
            nc.vector.tensor_copy(out=vcol, in_=vcol_i)
            d = m1.tile([128, S], F32, tag="d")
            nc.vector.tensor_scalar_sub(out=d, in0=vq_b, scalar1=vcol)
            adiff = m1.tile([128, S], F32, tag="adiff")
            nc.vector.tensor_scalar_mul(out=adiff, in0=d, scalar1=-1.0)
            nc.vector.tensor_tensor(out=adiff, in0=d, in1=adiff,
                                    op=mybir.AluOpType.max)
            m = m1.tile([128, S], F32, tag="m")
            nc.vector.tensor_scalar_max(out=m, in0=adiff, scalar1=1.0)
            nc.vector.tensor_scalar_mul(out=m, in0=m, scalar1=10.0 / 3.0)
            nc.vector.reciprocal(out=m, in_=m)
            c1 = m1.tile([128, S], F32, tag="c1")
            nc.vector.tensor_scalar(
                out=c1, in0=adiff, scalar1=1.0, scalar2=None,
                op0=mybir.AluOpType.is_equal)
            nc.vector.scalar_tensor_tensor(
                out=m, in0=c1, scalar=0.4, in1=m,
                op0=mybir.AluOpType.mult, op1=mybir.AluOpType.add)
            nc.vector.tensor_scalar(
                out=c1, in0=adiff, scalar1=0.0, scalar2=None,
                op0=mybir.AluOpType.is_equal)
            nc.vector.scalar_tensor_tensor(
                out=m, in0=c1, scalar=0.7, in1=m,
                op0=mybir.AluOpType.mult, op1=mybir.AluOpType.add)
            nc.vector.copy_predicated(out=m, mask=zq, data=ones_t)
            if vi == 0:
                nc.vector.memset(m[0:1, :], 1.0)
            for j in range(NQC):
                nc.sync.dma_start(
                    out=mdist[j][vi * 128:(vi + 1) * 128, :],
                    in_=m[:, j * QC:(j + 1) * QC])


def _build():
    nc = bacc.Bacc("TRN2", target_bir_lowering=False, debug=False,
                   num_devices=8)

    # ---- I/O ----
    xT = nc.dram_tensor("xT", [D, S], F32, kind="ExternalInput")
    x_sl = nc.dram_tensor("x_sl", [SS, D], F32, kind="ExternalInput")
    wqT = nc.dram_tensor("wqT", [D, DC], F32, kind="ExternalInput")
    wkT = nc.dram_tensor("wkT", [D, DC], F32, kind="ExternalInput")
    wvT = nc.dram_tensor("wvT", [D, DC], F32, kind="ExternalInput")
    woT = nc.dram_tensor("woT", [DC, D], F32, kind="ExternalInput")
    wpT = nc.dram_tensor("wpT", [D, D], F32, kind="ExternalInput")
    bq = nc.dram_tensor("bq", [128, 2], F32, kind="ExternalInput")
    bk = nc.dram_tensor("bk", [128, 2], F32, kind="ExternalInput")
    bv = nc.dram_tensor("bv", [DC], F32, kind="ExternalInput")
    bo4 = nc.dram_tensor("bo4", [128, NDCH], F32, kind="ExternalInput")
    bp = nc.dram_tensor("bp", [D], F32, kind="ExternalInput")
    gamma = nc.dram_tensor("gamma", [D], F32, kind="ExternalInput")
    beta = nc.dram_tensor("beta", [D], F32, kind="ExternalInput")
    vq = nc.dram_tensor("vq", [S], F32, kind="ExternalInput")
    vk_idx = nc.dram_tensor("vk_idx", [128, NKT], mybir.dt.int32,
                            kind="ExternalInput")
    out = nc.dram_tensor("out", [SS, D], F32, kind="ExternalOutput")
    dbg = {}
    if DEBUG:
        dbg["qT"] = nc.dram_tensor("dbg_qT", [128, 2, S], F32,
                                   kind="ExternalOutput")
        dbg["kT"] = nc.dram_tensor("dbg_kT", [128, 2, S], F32,
                                   kind="ExternalOutput")
        dbg["v"] = nc.dram_tensor("dbg_v", [128, NST, HPC, 65], F32,
                                  kind="ExternalOutput")
        dbg["mt"] = nc.dram_tensor("dbg_mt", [128, QC], F32,
                                   kind="ExternalOutput")
        dbg["et"] = nc.dram_tensor("dbg_et", [128, QC], F32,
                                   kind="ExternalOutput")
        dbg["ctx"] = nc.dram_tensor("dbg_ctx", [128, 2, S], F32,
                                    kind="ExternalOutput")
        dbg["part"] = nc.dram_tensor("dbg_part", [D, SS], F32,
                                     kind="ExternalOutput")
        dbg["rs"] = nc.dram_tensor("dbg_rs", [D, SS], F32,
                                   kind="ExternalOutput")
        dbg["opre"] = nc.dram_tensor("dbg_opre", [128, D], F32,
                                     kind="ExternalOutput")

    # ---- DRAM scratch ----
    mdist = [nc.dram_tensor(f"mdist{j}", [256, QC], F32) for j in range(NQC)]
    part_blk = nc.dram_tensor("part_blk", [4, D, SS], F32)
    rs_out = nc.dram_tensor("rs_out", [D, SS], F32)

    def bcast(ap, p=128):
        return AP(tensor=ap.tensor, offset=ap.offset,
                  ap=[[0, p]] + list(ap.ap))

    with tile.TileContext(nc) as tc:
        with tc.tile_pool(name="const", bufs=1) as const:
            # ---- small constants ----
            vki = const.tile([128, NKT], mybir.dt.int32)
            nc.sync.dma_start(out=vki, in_=vk_idx[:])
            bq_sb = const.tile([128, 2], F32)
            nc.sync.dma_start(out=bq_sb, in_=bq[:])
            bk_sb = const.tile([128, 2], F32)
            nc.sync.dma_start(out=bk_sb, in_=bk[:])
            bv_b = const.tile([128, DC], F32)
            nc.sync.dma_start(out=bv_b, in_=bcast(bv[:]))
            bo4_sb = const.tile([128, NDCH], F32)
            nc.sync.dma_start(out=bo4_sb, in_=bo4[:])
            eps_t = const.tile([128, 1], F32)
            nc.vector.memset(eps_t, EPS)
            ones64 = const.tile([128, 64], F32)
            nc.vector.memset(ones64, 1.0)

            _stage1_mask_rows(nc, tc, vq, mdist)

            with tc.tile_pool(name="pctx", bufs=1) as pctx:
                ctxT_sb = pctx.tile([128, 2, S], F32)

                with tc.tile_pool(name="pqkv", bufs=1) as pqkv:
                    qT_sb = pqkv.tile([128, 2, S], BF16)
                    kT_sb = pqkv.tile([128, 2, S], BF16)
                    v_sb = pqkv.tile([128, NST, HPC, 65], BF16)
                    nc.vector.memset(v_sb[:, :, :, 64:65], 1.0)

                    # ---- stage 2: projections qT, kT, v ----
                    with (
                        tc.tile_pool(name="wq", bufs=1) as wq_p,
                        tc.tile_pool(name="xt", bufs=8) as xt_p,
                        tc.tile_pool(name="pj", bufs=2, space="PSUM") as pj_p,
                    ):
                        wq_sb = wq_p.tile([128, NDCH, DC], F32)
                        wk_sb = wq_p.tile([128, NDCH, DC], F32)
                        wv_sb = wq_p.tile([128, NDCH, DC], F32)
                        nc.sync.dma_start(
                            out=wq_sb,
                            in_=wqT.ap().rearrange("(c p) o -> p c o", p=128))
                        nc.sync.dma_start(
                            out=wk_sb,
                            in_=wkT.ap().rearrange("(c p) o -> p c o", p=128))
                        nc.sync.dma_start(
                            out=wv_sb,
                            in_=wvT.ap().rearrange("(c p) o -> p c o", p=128))
                        xt_tiles = []
                        for ch in range(NDCH):
                            xt = xt_p.tile([128, S], F32, tag="xt",
                                           name=f"xt{ch}")
                            nc.sync.dma_start(
                                out=xt, in_=xT[ch * 128:(ch + 1) * 128, :])
                            xt_tiles.append(xt)
                        for j in range(2):
                            for sc in range(NQC):
                                ps_q = pj_p.tile([128, QC], F32, tag="psq")
                                ps_k = pj_p.tile([128, QC], F32, tag="psk")
                                for ch in range(NDCH):
                                    nc.tensor.matmul(
                                        ps_q,
                                        wq_sb[:, ch, j * 128:(j + 1) * 128],
                                        xt_tiles[ch][:, sc * QC:(sc + 1) * QC],
                                        start=(ch == 0), stop=(ch == NDCH - 1))
                                for ch in range(NDCH):
                                    nc.tensor.matmul(
                                        ps_k,
                                        wk_sb[:, ch, j * 128:(j + 1) * 128],
                                        xt_tiles[ch][:, sc * QC:(sc + 1) * QC],
                                        start=(ch == 0), stop=(ch == NDCH - 1))
                                nc.vector.tensor_scalar(
                                    out=qT_sb[:, j, sc * QC:(sc + 1) * QC],
                                    in0=ps_q, scalar1=bq_sb[:, j:j + 1],
                                    scalar2=SCALE,
                                    op0=mybir.AluOpType.add,
                                    op1=mybir.AluOpType.mult)
                                nc.vector.tensor_scalar(
                                    out=kT_sb[:, j, sc * QC:(sc + 1) * QC],
                                    in0=ps_k, scalar1=bk_sb[:, j:j + 1],
                                    scalar2=None, op0=mybir.AluOpType.add)
                        for st in range(NST):
                            ps_v = pj_p.tile([128, DC], F32, tag="psv")
                            for ch in range(NDCH):
                                nc.tensor.matmul(
                                    ps_v,
                                    xt_tiles[ch][:, st * 128:(st + 1) * 128],
                                    wv_sb[:, ch, :],
                                    start=(ch == 0), stop=(ch == NDCH - 1))
                            nc.vector.tensor_tensor(
                                out=v_sb[:, st, :, 0:64],
                                in0=ps_v.rearrange("p (h d) -> p h d", h=HPC),
                                in1=bv_b.rearrange("p (h d) -> p h d", h=HPC),
                                op=mybir.AluOpType.add)

                    if DEBUG:
                        nc.sync.dma_start(out=dbg["qT"][:], in_=qT_sb)
                        nc.sync.dma_start(out=dbg["kT"][:], in_=kT_sb)
                        nc.sync.dma_start(out=dbg["v"][:], in_=v_sb)

                    # ---- stage 3: attention ----
                    with (
                        tc.tile_pool(name="msk", bufs=NKT + 2) as msk_p,
                        tc.tile_pool(name="sp", bufs=3) as sp_p,
                        tc.tile_pool(name="et", bufs=4) as et_p,
                        tc.tile_pool(name="rc", bufs=4) as rc_p,
                        tc.tile_pool(name="pss", bufs=4, space="PSUM") as pss_p,
                        tc.tile_pool(name="psc", bufs=2, space="PSUM") as psc_p,
                        tc.tile_pool(name="rb", bufs=2, space="PSUM") as rb_p,
                    ):
                        for qc in range(NQC):
                            mtiles = []
                            for kt in range(NKT):
                                mt = msk_p.tile([128, QC], F32, tag="mt",
                                                name=f"mt{qc}_{kt}")
                                nc.gpsimd.indirect_dma_start(
                                    out=mt, out_offset=None, in_=mdist[qc][:],
                                    in_offset=bass.IndirectOffsetOnAxis(
                                        ap=vki[:, kt:kt + 1], axis=0))
                                mtiles.append(mt)
                            for pr in range(2):
                                ctx_ps = [
                                    psc_p.tile([65, QC], F32, tag="ctx",
                                               name=f"ctx_{qc}_{pr}_{i}")
                                    for i in range(2)]
                                for kt in range(NKT):
                                    ets = []
                                    for hh in range(2):
                                        lo = hh * 64
                                        ps_s = pss_p.tile([128, QC], F32,
                                                          tag="ps_s")
                                        nc.tensor.matmul(
                                            ps_s,
                                            kT_sb[lo:lo + 64, pr,
                                                  kt * 128:(kt + 1) * 128],
                                            qT_sb[lo:lo + 64, pr,
                                                  qc * QC:(qc + 1) * QC],
                                            start=True, stop=True)
                                        sp = sp_p.tile([128, QC], F32,
                                                       tag="sp")
                                        nc.vector.tensor_add(
                                            out=sp, in0=ps_s, in1=mtiles[kt])
                                        et = et_p.tile([128, QC], BF16,
                                                       tag="et")
                                        nc.scalar.activation(
                                            out=et, in_=sp,
                                            func=mybir.ActivationFunctionType.Exp)
                                        ets.append(et)
                                        if (DEBUG and qc == 0 and pr == 0
                                                and kt == 0 and hh == 0):
                                            nc.sync.dma_start(
                                                out=dbg["mt"][:],
                                                in_=mtiles[0])
                                            nc.sync.dma_start(
                                                out=dbg["et"][:], in_=et)
                                    for hh in range(2):
                                        h = 2 * pr + hh
                                        nc.tensor.matmul(
                                            ctx_ps[hh],
                                            v_sb[:, kt, h, :],
                                            ets[hh],
                                            start=(kt == 0),
                                            stop=(kt == NKT - 1))
                                cols = slice(qc * QC, (qc + 1) * QC)
                                for hh in range(2):
                                    # row 64 of ctx_ps = sum_k exp; rows
                                    # 0..63 = unnormalized ctxT [d, q].
                                    rec = rc_p.tile([128, QC], F32,
                                                    tag="rec")
                                    nc.vector.reciprocal(
                                        out=rec[64:65, :],
                                        in_=ctx_ps[hh][64:65, :])
                                    # outer-product broadcast of 1/sum to
                                    # partitions 0..63 via PE
                                    rb = rb_p.tile([64, QC], F32, tag="rb")
                                    nc.tensor.matmul(
                                        rb, ones64[64:65, :],
                                        rec[64:65, :],
                                        start=True, stop=True)
                                    rbs = rc_p.tile([64, QC], F32,
                                                    tag="rbs")
                                    nc.vector.tensor_copy(out=rbs, in_=rb)
                                    if hh == 0:
                                        nc.vector.tensor_tensor(
                                            out=ctxT_sb[0:64, pr, cols],
                                            in0=ctx_ps[hh][0:64, :],
                                            in1=rbs, op=mybir.AluOpType.mult)
                                    else:
                                        tmp = rc_p.tile([64, QC], F32,
                                                        tag="tmp")
                                        nc.vector.tensor_tensor(
                                            out=tmp,
                                            in0=ctx_ps[hh][0:64, :],
                                            in1=rbs, op=mybir.AluOpType.mult)
                                        nc.sync.dma_start(
                                            out=ctxT_sb[64:128, pr, cols],
                                            in_=tmp)

                # ---- stage 4: Wo partial -> part_blk ----
                if DEBUG:
                    nc.sync.dma_start(out=dbg["ctx"][:], in_=ctxT_sb)
                with (
                    tc.tile_pool(name="wo", bufs=1) as wo_p,
                    tc.tile_pool(name="ao", bufs=3) as ao_p,
                    tc.tile_pool(name="pst", bufs=4, space="PSUM") as pst_p,
                ):
                    wo_sb = wo_p.tile([128, 2, D], F32)
                    nc.sync.dma_start(
                        out=wo_sb,
                        in_=woT.ap().rearrange("(c p) o -> p c o", p=128))
                    for ot in range(NDCH):
                        for sc in range(NQC):
                            ps_a = pst_p.tile([128, QC], F32, tag="ps_a")
                            for pr in range(2):
                                nc.tensor.matmul(
                                    ps_a,
                                    wo_sb[:, pr, ot * 128:(ot + 1) * 128],
                                    ctxT_sb[:, pr, sc * QC:(sc + 1) * QC],
                                    start=(pr == 0), stop=(pr == 1))
                            ao = ao_p.tile([128, QC], F32, tag="ao")
                            nc.vector.tensor_scalar(
                                out=ao, in0=ps_a,
                                scalar1=bo4_sb[:, ot:ot + 1], scalar2=None,
                                op0=mybir.AluOpType.add)
                            nc.sync.dma_start(
                                out=part_blk[sc, ot * 128:(ot + 1) * 128, :],
                                in_=ao)

            # ---- stage 5: ReduceScatter ----
            if DEBUG:
                nc.sync.dma_start(out=dbg["part"][:], in_=part_blk[0])
            nc.gpsimd.collective_compute(
                "ReduceScatter", mybir.AluOpType.add, replica_groups=GROUPS,
                ins=[part_blk[:]], outs=[rs_out[:]])
            if DEBUG:
                nc.sync.dma_start(out=dbg["rs"][:], in_=rs_out[:])

            # ---- stage 6: Wp + residual + LayerNorm ----
            with (
                tc.tile_pool(name="wp", bufs=1) as wp_p,
                tc.tile_pool(name="rsx", bufs=1) as rsx_p,
                tc.tile_pool(name="o6", bufs=3) as o6_p,
                tc.tile_pool(name="st6", bufs=4) as st6_p,
                tc.tile_pool(name="ps6", bufs=4, space="PSUM") as ps6_p,
            ):
                wp_sb = wp_p.tile([128, NDCH, D], F32)
                nc.sync.dma_start(
                    out=wp_sb, in_=wpT.ap().rearrange("(c p) o -> p c o",
                                                      p=128))
                gamma_b = wp_p.tile([128, D], F32)
                nc.sync.dma_start(out=gamma_b, in_=bcast(gamma[:]))
                beta_b = wp_p.tile([128, D], F32)
                nc.sync.dma_start(out=beta_b, in_=bcast(beta[:]))
                bp_b = wp_p.tile([128, D], F32)
                nc.sync.dma_start(out=bp_b, in_=bcast(bp[:]))
                rsT_sb = rsx_p.tile([128, NDCH, SS], F32)
                nc.sync.dma_start(
                    out=rsT_sb,
                    in_=rs_out.ap().rearrange("(c p) s -> p c s", p=128))
                xpb = rsx_p.tile([128, 4, D], F32)
                nc.sync.dma_start(
                    out=xpb, in_=x_sl.ap().rearrange("(t p) o -> p t o",
                                                     p=128))
                for stl in range(4):
                    nc.vector.tensor_add(out=xpb[:, stl, :],
                                         in0=xpb[:, stl, :], in1=bp_b)
                for stl in range(4):
                    o_sb = o6_p.tile([128, D], F32, tag="o")
                    for nh in range(2):
                        ps_o = ps6_p.tile([128, 512], F32, tag="ps_o")
                        for ch in range(NDCH):
                            nc.tensor.matmul(
                                ps_o,
                                rsT_sb[:, ch, stl * 128:(stl + 1) * 128],
                                wp_sb[:, ch, nh * 512:(nh + 1) * 512],
                                start=(ch == 0), stop=(ch == NDCH - 1))
                        nc.vector.tensor_tensor(
                            out=o_sb[:, nh * 512:(nh + 1) * 512], in0=ps_o,
                            in1=xpb[:, stl, nh * 512:(nh + 1) * 512],
                            op=mybir.AluOpType.add)
                    if DEBUG and stl == 0:
                        nc.sync.dma_start(out=dbg["opre"][:], in_=o_sb)
                    stats = st6_p.tile([128, 2, 6], F32, tag="stats")
                    for i in range(2):
                        nc.vector.bn_stats(
                            out=stats[:, i, :],
                            in_=o_sb[:, i * 512:(i + 1) * 512])
                    mv = st6_p.tile([128, 2], F32, tag="mv")
                    nc.vector.bn_aggr(out=mv, in_=stats)
                    sd = st6_p.tile([128, 1], F32, tag="sd")
                    nc.scalar.activation(
                        out=sd, in_=mv[:, 1:2],
                        func=mybir.ActivationFunctionType.Sqrt,
                        bias=eps_t, scale=1.0)
                    rstd = st6_p.tile([128, 1], F32, tag="rstd")
                    nc.vector.reciprocal(out=rstd, in_=sd)
                    nc.vector.tensor_scalar(
                        out=o_sb, in0=o_sb, scalar1=mv[:, 0:1], scalar2=rstd,
                        op0=mybir.AluOpType.subtract, op1=mybir.AluOpType.mult)
                    nc.vector.tensor_mul(out=o_sb, in0=o_sb, in1=gamma_b)
                    nc.vector.tensor_add(out=o_sb, in0=o_sb, in1=beta_b)
                    nc.sync.dma_start(
                        out=out[stl * 128:(stl + 1) * 128, :], in_=o_sb)

    nc.compile()
    return nc


def _prep_inputs(x, verse_positions, W_q, b_q, W_k, b_k, W_v, b_v,
                 W_o, b_o, W_p, b_p, gamma, beta):
    f = np.float32
    x = np.asarray(x, f)
    vp = np.asarray(verse_positions)
    in_maps = []
    wpT = np.ascontiguousarray(np.asarray(W_p, f).T)
    for c in range(8):
        b, r = divmod(c, 4)
        sl = slice(DC * r, DC * (r + 1))
        vpb = vp[b].astype(np.int32)
        in_maps.append({
            "xT": np.ascontiguousarray(x[b].T),
            "x_sl": np.ascontiguousarray(x[b, SS * r:SS * (r + 1), :]),
            "wqT": np.ascontiguousarray(np.asarray(W_q, f)[sl, :].T),
            "wkT": np.ascontiguousarray(np.asarray(W_k, f)[sl, :].T),
            "wvT": np.ascontiguousarray(np.asarray(W_v, f)[sl, :].T),
            "woT": np.ascontiguousarray(np.asarray(W_o, f)[:, sl].T),
            "wpT": wpT,
            "bq": np.asarray(b_q, f)[sl].reshape(2, 128).T.copy(),
            "bk": np.asarray(b_k, f)[sl].reshape(2, 128).T.copy(),
            "bv": np.asarray(b_v, f)[sl].copy(),
            "bo4": (np.asarray(b_o, f) / 4.0).reshape(NDCH, 128).T.copy(),
            "bp": np.asarray(b_p, f).copy(),
            "gamma": np.asarray(gamma, f).copy(),
            "beta": np.asarray(beta, f).copy(),
            "vq": vpb.astype(f),
            "vk_idx": vpb.reshape(NKT, 128).T.copy(),
        })
    return in_maps


def kernel(**inputs):
    if "nc" not in _CACHE:
        _CACHE["nc"] = _build()
    nc = _CACHE["nc"]
    in_maps = _prep_inputs(**inputs)
    res = run_bass_kernel_spmd(nc, in_maps, core_ids=list(range(8)))
    _CACHE["last_res"] = res
    out = np.empty((B, S, D), np.float32)
    for c in range(8):
        b, r = divmod(c, 4)
        out[b, SS * r:SS * (r + 1), :] = res.results[c]["out"]
    return out


# revision 25
# speedup vs baseline: 2.1392x; 1.4319x over previous
"""CrossVerseAttention Trainium2 kernel.

Sharding: 8 cores = 2 batches x 4 head-groups. Core c handles batch c//4 and
heads [4*(c%4), 4*(c%4)+4). Attention scores are kept transposed [k, q] so the
cross-verse mask (<=200 distinct rows, verse values in [0,200)) can be added
per k-row tile via an indirect-DMA row gather; softmax runs without the max
subtraction (scores are O(1) here) with row sums produced by a ones column
appended to V inside the attn@V matmul. Partial attention outputs are
ReduceScattered (s-blocked, pre-transposed) across each 4-core batch group,
after which every core runs Wp + residual + LayerNorm on its own 512-row
slice.
"""
import os
import sys

sys.path.insert(0, "/opt/trn_rl_repo")

import numpy as np

DEBUG = bool(os.environ.get("CVK_DEBUG"))

import concourse.bacc as bacc
import concourse.bass as bass
import concourse.tile as tile
from concourse import mybir
from concourse.bass import AP
from concourse.bass_utils import run_bass_kernel_spmd
from concourse.masks import make_identity

B, S, D, H = 2, 2048, 1024, 16
DH = D // H            # 64
HPC = H // 4           # 4 heads per core
DC = HPC * DH          # 256 head dims per core
SS = S // 4            # 512 output rows per core
EPS = 1e-5
SCALE = 1.0 / float(np.sqrt(DH))
F32 = mybir.dt.float32
BF16 = mybir.dt.bfloat16
NQC = 4                # q chunks of 512
QC = S // NQC          # 512
NKT = S // 128         # 16 k tiles
NST = S // 128         # 16 s tiles
NDCH = D // 128        # 8 contraction chunks
GROUPS = [[0, 1, 2, 3], [4, 5, 6, 7]]

_CACHE = {}


def _stage1_mask_rows(nc, tc, vq, mdist):
    """Compute the <=256 distinct mask rows and store them to DRAM."""
    def bcast(ap, p=128):
        return AP(tensor=ap.tensor, offset=ap.offset,
                  ap=[[0, p]] + list(ap.ap))

    with (
        tc.tile_pool(name="m1", bufs=2) as m1,
        tc.tile_pool(name="m1c", bufs=1) as m1c,
        tc.tile_pool(name="m1s", bufs=2) as m1s,
    ):
        vq_b = m1c.tile([128, S], F32)
        nc.sync.dma_start(out=vq_b, in_=bcast(vq[:]))
        zq = m1c.tile([128, S], mybir.dt.int32)
        nc.vector.tensor_scalar(out=zq, in0=vq_b, scalar1=0.0, scalar2=None,
                                op0=mybir.AluOpType.is_equal)
        ones_t = m1c.tile([128, S], F32)
        nc.vector.memset(ones_t, 1.0)
        for vi in range(2):
            vcol_i = m1s.tile([128, 1], mybir.dt.int32)
            nc.gpsimd.iota(vcol_i, pattern=[[0, 1]], base=vi * 128,
                           channel_multiplier=1)
            vcol = m1s.tile([128, 1], F32)
            nc.vector.tensor_copy(out=vcol, in_=vcol_i)
            d = m1.tile([128, S], F32, tag="d")
            nc.vector.tensor_scalar_sub(out=d, in0=vq_b, scalar1=vcol)
            adiff = m1.tile([128, S], F32, tag="adiff")
            nc.vector.tensor_scalar_mul(out=adiff, in0=d, scalar1=-1.0)
            nc.vector.tensor_tensor(out=adiff, in0=d, in1=adiff,
                                    op=mybir.AluOpType.max)
            m = m1.tile([128, S], F32, tag="m")
            nc.vector.tensor_scalar_max(out=m, in0=adiff, scalar1=1.0)
            nc.vector.tensor_scalar_mul(out=m, in0=m, scalar1=10.0 / 3.0)
            nc.vector.reciprocal(out=m, in_=m)
            c1 = m1.tile([128, S], F32, tag="c1")
            nc.vector.tensor_scalar(
                out=c1, in0=adiff, scalar1=1.0, scalar2=None,
                op0=mybir.AluOpType.is_equal)
            nc.vector.scalar_tensor_tensor(
                out=m, in0=c1, scalar=0.4, in1=m,
                op0=mybir.AluOpType.mult, op1=mybir.AluOpType.add)
            nc.vector.tensor_scalar(
                out=c1, in0=adiff, scalar1=0.0, scalar2=None,
                op0=mybir.AluOpType.is_equal)
            nc.vector.scalar_tensor_tensor(
                out=m, in0=c1, scalar=0.7, in1=m,
                op0=mybir.AluOpType.mult, op1=mybir.AluOpType.add)
            nc.vector.copy_predicated(out=m, mask=zq, data=ones_t)
            if vi == 0:
                nc.vector.memset(m[0:1, :], 1.0)
            for j in range(NQC):
                nc.sync.dma_start(
                    out=mdist[j][vi * 128:(vi + 1) * 128, :],
                    in_=m[:, j * QC:(j + 1) * QC])


def _build():
    nc = bacc.Bacc("TRN2", target_bir_lowering=False, debug=False,
                   num_devices=8)

    # ---- I/O ----
    xT = nc.dram_tensor("xT", [D, S], BF16, kind="ExternalInput")
    x_sl = nc.dram_tensor("x_sl", [SS, D], F32, kind="ExternalInput")
    wqT = nc.dram_tensor("wqT", [D, DC], BF16, kind="ExternalInput")
    wkT = nc.dram_tensor("wkT", [D, DC], BF16, kind="ExternalInput")
    wvT = nc.dram_tensor("wvT", [D, DC], BF16, kind="ExternalInput")
    woT = nc.dram_tensor("woT", [DC, D], F32, kind="ExternalInput")
    wpT = nc.dram_tensor("wpT", [D, D], F32, kind="ExternalInput")
    bq = nc.dram_tensor("bq", [128, 2], F32, kind="ExternalInput")
    bk = nc.dram_tensor("bk", [128, 2], F32, kind="ExternalInput")
    bv = nc.dram_tensor("bv", [DC], F32, kind="ExternalInput")
    bo4 = nc.dram_tensor("bo4", [128, NDCH], F32, kind="ExternalInput")
    bp = nc.dram_tensor("bp", [D], F32, kind="ExternalInput")
    gamma = nc.dram_tensor("gamma", [D], F32, kind="ExternalInput")
    beta = nc.dram_tensor("beta", [D], F32, kind="ExternalInput")
    vq = nc.dram_tensor("vq", [S], F32, kind="ExternalInput")
    vk_idx = nc.dram_tensor("vk_idx", [128, NKT], mybir.dt.int32,
                            kind="ExternalInput")
    out = nc.dram_tensor("out", [SS, D], F32, kind="ExternalOutput")
    dbg = {}
    if DEBUG:
        dbg["qT"] = nc.dram_tensor("dbg_qT", [128, 2, S], F32,
                                   kind="ExternalOutput")
        dbg["kT"] = nc.dram_tensor("dbg_kT", [128, 2, S], F32,
                                   kind="ExternalOutput")
        dbg["v"] = nc.dram_tensor("dbg_v", [128, NST, HPC, 65], F32,
                                  kind="ExternalOutput")
        dbg["mt"] = nc.dram_tensor("dbg_mt", [128, QC], F32,
                                   kind="ExternalOutput")
        dbg["et"] = nc.dram_tensor("dbg_et", [128, QC], F32,
                                   kind="ExternalOutput")
        dbg["ctx"] = nc.dram_tensor("dbg_ctx", [128, 2, S], F32,
                                    kind="ExternalOutput")
        dbg["part"] = nc.dram_tensor("dbg_part", [D, SS], F32,
                                     kind="ExternalOutput")
        dbg["rs"] = nc.dram_tensor("dbg_rs", [D, SS], F32,
                                   kind="ExternalOutput")
        dbg["opre"] = nc.dram_tensor("dbg_opre", [128, D], F32,
                                     kind="ExternalOutput")

    # ---- DRAM scratch ----
    mdist = [nc.dram_tensor(f"mdist{j}", [256, QC], F32) for j in range(NQC)]
    # chunk-major so each ReduceScatter operates on a contiguous block
    part_blk = nc.dram_tensor("part_blk", [NDCH // 2, 4, 256, SS], F32)
    rs_out = nc.dram_tensor("rs_out", [NDCH // 2, 256, SS], F32)

    def bcast(ap, p=128):
        return AP(tensor=ap.tensor, offset=ap.offset,
                  ap=[[0, p]] + list(ap.ap))

    with tile.TileContext(nc) as tc:
        with tc.tile_pool(name="const", bufs=1) as const:
            # ---- small constants ----
            vki = const.tile([128, NKT], mybir.dt.int32)
            nc.sync.dma_start(out=vki, in_=vk_idx[:])
            bq_sb = const.tile([128, 2], F32)
            nc.sync.dma_start(out=bq_sb, in_=bq[:])
            bk_sb = const.tile([128, 2], F32)
            nc.sync.dma_start(out=bk_sb, in_=bk[:])
            bv_b = const.tile([128, DC], F32)
            nc.sync.dma_start(out=bv_b, in_=bcast(bv[:]))
            bo4_sb = const.tile([128, NDCH], F32)
            nc.sync.dma_start(out=bo4_sb, in_=bo4[:])
            eps_t = const.tile([128, 1], F32)
            nc.vector.memset(eps_t, EPS)
            ones64 = const.tile([128, 64], F32)
            nc.vector.memset(ones64, 1.0)

            _stage1_mask_rows(nc, tc, vq, mdist)

            with tc.tile_pool(name="pctx", bufs=1) as pctx:
                ctxT_sb = pctx.tile([128, 2, S], F32)

                with tc.tile_pool(name="pqkv", bufs=1) as pqkv:
                    qT_sb = pqkv.tile([128, 2, S], BF16)
                    kT_sb = pqkv.tile([128, 2, S], BF16)
                    v_sb = pqkv.tile([128, NST, HPC, 65], BF16)
                    nc.vector.memset(v_sb[:, :, :, 64:65], 1.0)

                    # ---- stage 2: projections qT, kT, v ----
                    with (
                        tc.tile_pool(name="wq", bufs=1) as wq_p,
                        tc.tile_pool(name="xt", bufs=8) as xt_p,
                        tc.tile_pool(name="pj", bufs=2, space="PSUM") as pj_p,
                    ):
                        wq_sb = wq_p.tile([128, NDCH, DC], BF16)
                        wk_sb = wq_p.tile([128, NDCH, DC], BF16)
                        wv_sb = wq_p.tile([128, NDCH, DC], BF16)
                        nc.sync.dma_start(
                            out=wq_sb,
                            in_=wqT.ap().rearrange("(c p) o -> p c o", p=128))
                        nc.sync.dma_start(
                            out=wk_sb,
                            in_=wkT.ap().rearrange("(c p) o -> p c o", p=128))
                        nc.sync.dma_start(
                            out=wv_sb,
                            in_=wvT.ap().rearrange("(c p) o -> p c o", p=128))
                        xt_tiles = []
                        for ch in range(NDCH):
                            xt = xt_p.tile([128, S], BF16, tag="xt",
                                           name=f"xt{ch}")
                            nc.sync.dma_start(
                                out=xt, in_=xT[ch * 128:(ch + 1) * 128, :])
                            xt_tiles.append(xt)
                        for j in range(2):
                            for sc in range(NQC):
                                ps_q = pj_p.tile([128, QC], F32, tag="psq")
                                ps_k = pj_p.tile([128, QC], F32, tag="psk")
                                for ch in range(NDCH):
                                    nc.tensor.matmul(
                                        ps_q,
                                        wq_sb[:, ch, j * 128:(j + 1) * 128],
                                        xt_tiles[ch][:, sc * QC:(sc + 1) * QC],
                                        start=(ch == 0), stop=(ch == NDCH - 1))
                                for ch in range(NDCH):
                                    nc.tensor.matmul(
                                        ps_k,
                                        wk_sb[:, ch, j * 128:(j + 1) * 128],
                                        xt_tiles[ch][:, sc * QC:(sc + 1) * QC],
                                        start=(ch == 0), stop=(ch == NDCH - 1))
                                nc.vector.tensor_scalar(
                                    out=qT_sb[:, j, sc * QC:(sc + 1) * QC],
                                    in0=ps_q, scalar1=bq_sb[:, j:j + 1],
                                    scalar2=SCALE,
                                    op0=mybir.AluOpType.add,
                                    op1=mybir.AluOpType.mult)
                                nc.vector.tensor_scalar(
                                    out=kT_sb[:, j, sc * QC:(sc + 1) * QC],
                                    in0=ps_k, scalar1=bk_sb[:, j:j + 1],
                                    scalar2=None, op0=mybir.AluOpType.add)
                        for st in range(NST):
                            ps_v = pj_p.tile([128, DC], F32, tag="psv")
                            for ch in range(NDCH):
                                nc.tensor.matmul(
                                    ps_v,
                                    xt_tiles[ch][:, st * 128:(st + 1) * 128],
                                    wv_sb[:, ch, :],
                                    start=(ch == 0), stop=(ch == NDCH - 1))
                            nc.vector.tensor_tensor(
                                out=v_sb[:, st, :, 0:64],
                                in0=ps_v.rearrange("p (h d) -> p h d", h=HPC),
                                in1=bv_b.rearrange("p (h d) -> p h d", h=HPC),
                                op=mybir.AluOpType.add)

                    if DEBUG:
                        nc.sync.dma_start(out=dbg["qT"][:], in_=qT_sb)
                        nc.sync.dma_start(out=dbg["kT"][:], in_=kT_sb)
                        nc.sync.dma_start(out=dbg["v"][:], in_=v_sb)

                    # ---- stage 3: attention ----
                    with (
                        tc.tile_pool(name="msk", bufs=NKT + 2) as msk_p,
                        tc.tile_pool(name="sp", bufs=3) as sp_p,
                        tc.tile_pool(name="et", bufs=4) as et_p,
                        tc.tile_pool(name="rc", bufs=4) as rc_p,
                        tc.tile_pool(name="pss", bufs=4, space="PSUM") as pss_p,
                        tc.tile_pool(name="psc", bufs=2, space="PSUM") as psc_p,
                        tc.tile_pool(name="rb", bufs=2, space="PSUM") as rb_p,
                    ):
                        for qc in range(NQC):
                            mtiles = []
                            for kt in range(NKT):
                                mt = msk_p.tile([128, QC], F32, tag="mt",
                                                name=f"mt{qc}_{kt}")
                                nc.gpsimd.indirect_dma_start(
                                    out=mt, out_offset=None, in_=mdist[qc][:],
                                    in_offset=bass.IndirectOffsetOnAxis(
                                        ap=vki[:, kt:kt + 1], axis=0))
                                mtiles.append(mt)
                            for pr in range(2):
                                ctx_ps = [
                                    psc_p.tile([65, QC], F32, tag="ctx",
                                               name=f"ctx_{qc}_{pr}_{i}")
                                    for i in range(2)]
                                for kt in range(NKT):
                                    sp = sp_p.tile([128, 2, QC], F32,
                                                   tag="sp")
                                    for hh in range(2):
                                        lo = hh * 64
                                        ps_s = pss_p.tile([128, QC], F32,
                                                          tag="ps_s")
                                        nc.tensor.matmul(
                                            ps_s,
                                            kT_sb[lo:lo + 64, pr,
                                                  kt * 128:(kt + 1) * 128],
                                            qT_sb[lo:lo + 64, pr,
                                                  qc * QC:(qc + 1) * QC],
                                            start=True, stop=True)
                                        nc.vector.tensor_add(
                                            out=sp[:, hh, :], in0=ps_s,
                                            in1=mtiles[kt])
                                    et = et_p.tile([128, 2, QC], BF16,
                                                   tag="et")
                                    nc.scalar.activation(
                                        out=et, in_=sp,
                                        func=mybir.ActivationFunctionType.Exp)
                                    if DEBUG and qc == 0 and pr == 0 and kt == 0:
                                        nc.sync.dma_start(out=dbg["mt"][:],
                                                          in_=mtiles[0])
                                        nc.sync.dma_start(out=dbg["et"][:],
                                                          in_=et[:, 0, :])
                                    for hh in range(2):
                                        h = 2 * pr + hh
                                        nc.tensor.matmul(
                                            ctx_ps[hh],
                                            v_sb[:, kt, h, :],
                                            et[:, hh, :],
                                            start=(kt == 0),
                                            stop=(kt == NKT - 1))
                                cols = slice(qc * QC, (qc + 1) * QC)
                                for hh in range(2):
                                    # row 64 of ctx_ps = sum_k exp; rows
                                    # 0..63 = unnormalized ctxT [d, q].
                                    # Reciprocal over all 65 rows: only row
                                    # 64 (1/sums) is used, but running all
                                    # lanes beats a 1-lane op by ~5x.
                                    rec = rc_p.tile([65, QC], F32,
                                                    tag="rec")
                                    nc.vector.reciprocal(
                                        out=rec, in_=ctx_ps[hh][0:65, :])
                                    # outer-product broadcast of 1/sum to
                                    # partitions 0..63 via PE
                                    rb = rb_p.tile([64, QC], F32, tag="rb")
                                    nc.tensor.matmul(
                                        rb, ones64[64:65, :],
                                        rec[64:65, :],
                                        start=True, stop=True)
                                    rbs = rc_p.tile([64, QC], F32,
                                                    tag="rbs")
                                    nc.vector.tensor_copy(out=rbs, in_=rb)
                                    if hh == 0:
                                        nc.vector.tensor_tensor(
                                            out=ctxT_sb[0:64, pr, cols],
                                            in0=ctx_ps[hh][0:64, :],
                                            in1=rbs, op=mybir.AluOpType.mult)
                                    else:
                                        tmp = rc_p.tile([64, QC], F32,
                                                        tag="tmp")
                                        nc.vector.tensor_tensor(
                                            out=tmp,
                                            in0=ctx_ps[hh][0:64, :],
                                            in1=rbs, op=mybir.AluOpType.mult)
                                        nc.sync.dma_start(
                                            out=ctxT_sb[64:128, pr, cols],
                                            in_=tmp)

                # ---- stage 4: Wo partial -> part_blk ----
                if DEBUG:
                    nc.sync.dma_start(out=dbg["ctx"][:], in_=ctxT_sb)
                with (
                    tc.tile_pool(name="wo", bufs=1) as wo_p,
                    tc.tile_pool(name="ao", bufs=3) as ao_p,
                    tc.tile_pool(name="pst", bufs=4, space="PSUM") as pst_p,
                ):
                    wo_sb = wo_p.tile([128, 2, D], F32)
                    nc.sync.dma_start(
                        out=wo_sb,
                        in_=woT.ap().rearrange("(c p) o -> p c o", p=128))
                    for ot in range(NDCH):
                        for sc in range(NQC):
                            ps_a = pst_p.tile([128, QC], F32, tag="ps_a")
                            for pr in range(2):
                                nc.tensor.matmul(
                                    ps_a,
                                    wo_sb[:, pr, ot * 128:(ot + 1) * 128],
                                    ctxT_sb[:, pr, sc * QC:(sc + 1) * QC],
                                    start=(pr == 0), stop=(pr == 1))
                            ao = ao_p.tile([128, QC], F32, tag="ao")
                            nc.vector.tensor_scalar(
                                out=ao, in0=ps_a,
                                scalar1=bo4_sb[:, ot:ot + 1], scalar2=None,
                                op0=mybir.AluOpType.add)
                            nc.sync.dma_start(
                                out=part_blk[ot // 2, sc,
                                             (ot % 2) * 128:
                                             (ot % 2 + 1) * 128, :],
                                in_=ao)
                        if ot % 2 == 1:
                            i = ot // 2
                            nc.gpsimd.collective_compute(
                                "ReduceScatter", mybir.AluOpType.add,
                                replica_groups=GROUPS,
                                ins=[part_blk[i]],
                                outs=[rs_out[i]])

            # ---- stage 5: ReduceScatter issued chunked inside stage 4 ----
            if DEBUG:
                nc.sync.dma_start(
                    out=dbg["part"][:],
                    in_=part_blk.ap().rearrange("i g (t p) s -> g (i t p) s",
                                                p=128)[0])
                nc.sync.dma_start(
                    out=dbg["rs"][:],
                    in_=rs_out.ap().rearrange("i t s -> (i t) s"))

            # ---- stage 6: Wp + residual + LayerNorm ----
            with (
                tc.tile_pool(name="wp", bufs=1) as wp_p,
                tc.tile_pool(name="rsx", bufs=1) as rsx_p,
                tc.tile_pool(name="o6", bufs=3) as o6_p,
                tc.tile_pool(name="st6", bufs=4) as st6_p,
                tc.tile_pool(name="ps6", bufs=4, space="PSUM") as ps6_p,
            ):
                wp_sb = wp_p.tile([128, NDCH, D], F32)
                nc.sync.dma_start(
                    out=wp_sb, in_=wpT.ap().rearrange("(c p) o -> p c o",
                                                      p=128))
                gamma_b = wp_p.tile([128, D], F32)
                nc.sync.dma_start(out=gamma_b, in_=bcast(gamma[:]))
                beta_b = wp_p.tile([128, D], F32)
                nc.sync.dma_start(out=beta_b, in_=bcast(beta[:]))
                bp_b = wp_p.tile([128, D], F32)
                nc.sync.dma_start(out=bp_b, in_=bcast(bp[:]))
                rsT_sb = rsx_p.tile([128, NDCH, SS], F32)
                for i in range(NDCH // 2):
                    nc.sync.dma_start(
                        out=rsT_sb[:, 2 * i:2 * i + 2, :],
                        in_=rs_out[i].rearrange("(c p) s -> p c s", p=128))
                xpb = rsx_p.tile([128, 4, D], F32)
                nc.sync.dma_start(
                    out=xpb, in_=x_sl.ap().rearrange("(t p) o -> p t o",
                                                     p=128))
                for stl in range(4):
                    nc.vector.tensor_add(out=xpb[:, stl, :],
                                         in0=xpb[:, stl, :], in1=bp_b)
                for stl in range(4):
                    o_sb = o6_p.tile([128, D], F32, tag="o")
                    for nh in range(2):
                        ps_o = ps6_p.tile([128, 512], F32, tag="ps_o")
                        for ch in range(NDCH):
                            nc.tensor.matmul(
                                ps_o,
                                rsT_sb[:, ch, stl * 128:(stl + 1) * 128],
                                wp_sb[:, ch, nh * 512:(nh + 1) * 512],
                                start=(ch == 0), stop=(ch == NDCH - 1))
                        nc.vector.tensor_tensor(
                            out=o_sb[:, nh * 512:(nh + 1) * 512], in0=ps_o,
                            in1=xpb[:, stl, nh * 512:(nh + 1) * 512],
                            op=mybir.AluOpType.add)
                    if DEBUG and stl == 0:
                        nc.sync.dma_start(out=dbg["opre"][:], in_=o_sb)
                    stats = st6_p.tile([128, 2, 6], F32, tag="stats")
                    for i in range(2):
                        nc.vector.bn_stats(
                            out=stats[:, i, :],
                            in_=o_sb[:, i * 512:(i + 1) * 512])
                    mv = st6_p.tile([128, 2], F32, tag="mv")
                    nc.vector.bn_aggr(out=mv, in_=stats)
                    sd = st6_p.tile([128, 1], F32, tag="sd")
                    nc.scalar.activation(
                        out=sd, in_=mv[:, 1:2],
                        func=mybir.ActivationFunctionType.Sqrt,
                        bias=eps_t, scale=1.0)
                    rstd = st6_p.tile([128, 1], F32, tag="rstd")
                    nc.vector.reciprocal(out=rstd, in_=sd)
                    nc.vector.tensor_scalar(
                        out=o_sb, in0=o_sb, scalar1=mv[:, 0:1], scalar2=rstd,
                        op0=mybir.AluOpType.subtract, op1=mybir.AluOpType.mult)
                    nc.vector.tensor_mul(out=o_sb, in0=o_sb, in1=gamma_b)
                    nc.vector.tensor_add(out=o_sb, in0=o_sb, in1=beta_b)
                    nc.sync.dma_start(
                        out=out[stl * 128:(stl + 1) * 128, :], in_=o_sb)

    nc.compile()
    return nc


def _prep_inputs(x, verse_positions, W_q, b_q, W_k, b_k, W_v, b_v,
                 W_o, b_o, W_p, b_p, gamma, beta):
    import ml_dtypes
    f = np.float32
    bf = ml_dtypes.bfloat16
    x = np.asarray(x, f)
    vp = np.asarray(verse_positions)
    in_maps = []
    wpT = np.ascontiguousarray(np.asarray(W_p, f).T)
    xTb = [np.ascontiguousarray(x[b].T).astype(bf) for b in range(B)]
    for c in range(8):
        b, r = divmod(c, 4)
        sl = slice(DC * r, DC * (r + 1))
        vpb = vp[b].astype(np.int32)
        in_maps.append({
            "xT": xTb[b],
            "x_sl": np.ascontiguousarray(x[b, SS * r:SS * (r + 1), :]),
            "wqT": np.ascontiguousarray(np.asarray(W_q, f)[sl, :].T).astype(bf),
            "wkT": np.ascontiguousarray(np.asarray(W_k, f)[sl, :].T).astype(bf),
            "wvT": np.ascontiguousarray(np.asarray(W_v, f)[sl, :].T).astype(bf),
            "woT": np.ascontiguousarray(np.asarray(W_o, f)[:, sl].T),
            "wpT": wpT,
            "bq": np.asarray(b_q, f)[sl].reshape(2, 128).T.copy(),
            "bk": np.asarray(b_k, f)[sl].reshape(2, 128).T.copy(),
            "bv": np.asarray(b_v, f)[sl].copy(),
            "bo4": (np.asarray(b_o, f) / 4.0).reshape(NDCH, 128).T.copy(),
            "bp": np.asarray(b_p, f).copy(),
            "gamma": np.asarray(gamma, f).copy(),
            "beta": np.asarray(beta, f).copy(),
            "vq": vpb.astype(f),
            "vk_idx": vpb.reshape(NKT, 128).T.copy(),
        })
    return in_maps


def kernel(**inputs):
    if "nc" not in _CACHE:
        _CACHE["nc"] = _build()
    nc = _CACHE["nc"]
    in_maps = _prep_inputs(**inputs)
    res = run_bass_kernel_spmd(nc, in_maps, core_ids=list(range(8)))
    _CACHE["last_res"] = res
    out = np.empty((B, S, D), np.float32)
    for c in range(8):
        b, r = divmod(c, 4)
        out[b, SS * r:SS * (r + 1), :] = res.results[c]["out"]
    return out


# revision 26
# speedup vs baseline: 2.1567x; 1.0082x over previous
"""CrossVerseAttention Trainium2 kernel.

Sharding: 8 cores = 2 batches x 4 head-groups. Core c handles batch c//4 and
heads [4*(c%4), 4*(c%4)+4). Attention scores are kept transposed [k, q] so the
cross-verse mask (<=200 distinct rows, verse values in [0,200)) can be added
per k-row tile via an indirect-DMA row gather; softmax runs without the max
subtraction (scores are O(1) here) with row sums produced by a ones column
appended to V inside the attn@V matmul. Partial attention outputs are
ReduceScattered (s-blocked, pre-transposed) across each 4-core batch group,
after which every core runs Wp + residual + LayerNorm on its own 512-row
slice.
"""
import os
import sys

sys.path.insert(0, "/opt/trn_rl_repo")

import numpy as np

DEBUG = bool(os.environ.get("CVK_DEBUG"))

import concourse.bacc as bacc
import concourse.bass as bass
import concourse.tile as tile
from concourse import mybir
from concourse.bass import AP
from concourse.bass_utils import run_bass_kernel_spmd
from concourse.masks import make_identity

B, S, D, H = 2, 2048, 1024, 16
DH = D // H            # 64
HPC = H // 4           # 4 heads per core
DC = HPC * DH          # 256 head dims per core
SS = S // 4            # 512 output rows per core
EPS = 1e-5
SCALE = 1.0 / float(np.sqrt(DH))
F32 = mybir.dt.float32
BF16 = mybir.dt.bfloat16
NQC = 4                # q chunks of 512
QC = S // NQC          # 512
NKT = S // 128         # 16 k tiles
NST = S // 128         # 16 s tiles
NDCH = D // 128        # 8 contraction chunks
GROUPS = [[0, 1, 2, 3], [4, 5, 6, 7]]

_CACHE = {}


def _stage1_mask_rows(nc, tc, vq, mdist):
    """Compute the <=256 distinct mask rows and store them to DRAM."""
    def bcast(ap, p=128):
        return AP(tensor=ap.tensor, offset=ap.offset,
                  ap=[[0, p]] + list(ap.ap))

    with (
        tc.tile_pool(name="m1", bufs=2) as m1,
        tc.tile_pool(name="m1c", bufs=1) as m1c,
        tc.tile_pool(name="m1s", bufs=2) as m1s,
    ):
        vq_b = m1c.tile([128, S], F32)
        nc.sync.dma_start(out=vq_b, in_=bcast(vq[:]))
        zq = m1c.tile([128, S], mybir.dt.int32)
        nc.vector.tensor_scalar(out=zq, in0=vq_b, scalar1=0.0, scalar2=None,
                                op0=mybir.AluOpType.is_equal)
        ones_t = m1c.tile([128, S], F32)
        nc.vector.memset(ones_t, 1.0)
        for vi in range(2):
            vcol_i = m1s.tile([128, 1], mybir.dt.int32)
            nc.gpsimd.iota(vcol_i, pattern=[[0, 1]], base=vi * 128,
                           channel_multiplier=1)
            vcol = m1s.tile([128, 1], F32)
            nc.vector.tensor_copy(out=vcol, in_=vcol_i)
            d = m1.tile([128, S], F32, tag="d")
            nc.vector.tensor_scalar_sub(out=d, in0=vq_b, scalar1=vcol)
            adiff = m1.tile([128, S], F32, tag="adiff")
            nc.vector.tensor_scalar_mul(out=adiff, in0=d, scalar1=-1.0)
            nc.vector.tensor_tensor(out=adiff, in0=d, in1=adiff,
                                    op=mybir.AluOpType.max)
            m = m1.tile([128, S], F32, tag="m")
            nc.vector.tensor_scalar_max(out=m, in0=adiff, scalar1=1.0)
            nc.vector.tensor_scalar_mul(out=m, in0=m, scalar1=10.0 / 3.0)
            nc.vector.reciprocal(out=m, in_=m)
            c1 = m1.tile([128, S], F32, tag="c1")
            nc.vector.tensor_scalar(
                out=c1, in0=adiff, scalar1=1.0, scalar2=None,
                op0=mybir.AluOpType.is_equal)
            nc.vector.scalar_tensor_tensor(
                out=m, in0=c1, scalar=0.4, in1=m,
                op0=mybir.AluOpType.mult, op1=mybir.AluOpType.add)
            nc.vector.tensor_scalar(
                out=c1, in0=adiff, scalar1=0.0, scalar2=None,
                op0=mybir.AluOpType.is_equal)
            nc.vector.scalar_tensor_tensor(
                out=m, in0=c1, scalar=0.7, in1=m,
                op0=mybir.AluOpType.mult, op1=mybir.AluOpType.add)
            nc.vector.copy_predicated(out=m, mask=zq, data=ones_t)
            if vi == 0:
                nc.vector.memset(m[0:1, :], 1.0)
            for j in range(NQC):
                nc.sync.dma_start(
                    out=mdist[j][vi * 128:(vi + 1) * 128, :],
                    in_=m[:, j * QC:(j + 1) * QC])


def _build():
    nc = bacc.Bacc("TRN2", target_bir_lowering=False, debug=False,
                   num_devices=8)

    # ---- I/O ----
    xT = nc.dram_tensor("xT", [D, S], BF16, kind="ExternalInput")
    x_sl = nc.dram_tensor("x_sl", [SS, D], F32, kind="ExternalInput")
    wqT = nc.dram_tensor("wqT", [D, DC], BF16, kind="ExternalInput")
    wkT = nc.dram_tensor("wkT", [D, DC], BF16, kind="ExternalInput")
    wvT = nc.dram_tensor("wvT", [D, DC], BF16, kind="ExternalInput")
    woT = nc.dram_tensor("woT", [DC, D], F32, kind="ExternalInput")
    wpT = nc.dram_tensor("wpT", [D, D], F32, kind="ExternalInput")
    bq = nc.dram_tensor("bq", [128, 2], F32, kind="ExternalInput")
    bk = nc.dram_tensor("bk", [128, 2], F32, kind="ExternalInput")
    bv = nc.dram_tensor("bv", [DC], F32, kind="ExternalInput")
    bo4 = nc.dram_tensor("bo4", [128, NDCH], F32, kind="ExternalInput")
    bp = nc.dram_tensor("bp", [D], F32, kind="ExternalInput")
    gamma = nc.dram_tensor("gamma", [D], F32, kind="ExternalInput")
    beta = nc.dram_tensor("beta", [D], F32, kind="ExternalInput")
    vq = nc.dram_tensor("vq", [S], F32, kind="ExternalInput")
    vk_idx = nc.dram_tensor("vk_idx", [128, NKT], mybir.dt.int32,
                            kind="ExternalInput")
    out = nc.dram_tensor("out", [SS, D], F32, kind="ExternalOutput")
    dbg = {}
    if DEBUG:
        dbg["qT"] = nc.dram_tensor("dbg_qT", [128, 2, S], F32,
                                   kind="ExternalOutput")
        dbg["kT"] = nc.dram_tensor("dbg_kT", [128, 2, S], F32,
                                   kind="ExternalOutput")
        dbg["v"] = nc.dram_tensor("dbg_v", [128, NST, HPC, 65], F32,
                                  kind="ExternalOutput")
        dbg["mt"] = nc.dram_tensor("dbg_mt", [128, QC], F32,
                                   kind="ExternalOutput")
        dbg["et"] = nc.dram_tensor("dbg_et", [128, QC], F32,
                                   kind="ExternalOutput")
        dbg["ctx"] = nc.dram_tensor("dbg_ctx", [128, 2, S], F32,
                                    kind="ExternalOutput")
        dbg["part"] = nc.dram_tensor("dbg_part", [D, SS], F32,
                                     kind="ExternalOutput")
        dbg["rs"] = nc.dram_tensor("dbg_rs", [D, SS], F32,
                                   kind="ExternalOutput")
        dbg["opre"] = nc.dram_tensor("dbg_opre", [128, D], F32,
                                     kind="ExternalOutput")

    # ---- DRAM scratch ----
    mdist = [nc.dram_tensor(f"mdist{j}", [256, QC], F32) for j in range(NQC)]
    # one tensor pair per RS chunk: separate tensors give per-chunk
    # dependency tracking so each ReduceScatter can fire as soon as its
    # own o-rows are written (whole-tensor deps would serialize them all)
    part_blk = [nc.dram_tensor(f"part_blk{i}", [4, 256, SS], F32)
                for i in range(NDCH // 2)]
    rs_out = [nc.dram_tensor(f"rs_out{i}", [256, SS], F32)
              for i in range(NDCH // 2)]

    def bcast(ap, p=128):
        return AP(tensor=ap.tensor, offset=ap.offset,
                  ap=[[0, p]] + list(ap.ap))

    with tile.TileContext(nc) as tc:
        with tc.tile_pool(name="const", bufs=1) as const:
            # ---- small constants ----
            vki = const.tile([128, NKT], mybir.dt.int32)
            nc.sync.dma_start(out=vki, in_=vk_idx[:])
            bq_sb = const.tile([128, 2], F32)
            nc.sync.dma_start(out=bq_sb, in_=bq[:])
            bk_sb = const.tile([128, 2], F32)
            nc.sync.dma_start(out=bk_sb, in_=bk[:])
            bv_b = const.tile([128, DC], F32)
            nc.sync.dma_start(out=bv_b, in_=bcast(bv[:]))
            bo4_sb = const.tile([128, NDCH], F32)
            nc.sync.dma_start(out=bo4_sb, in_=bo4[:])
            eps_t = const.tile([128, 1], F32)
            nc.vector.memset(eps_t, EPS)
            ones64 = const.tile([128, 64], F32)
            nc.vector.memset(ones64, 1.0)

            _stage1_mask_rows(nc, tc, vq, mdist)

            with tc.tile_pool(name="pctx", bufs=1) as pctx:
                ctxT_sb = pctx.tile([128, 2, S], F32)

                with tc.tile_pool(name="pqkv", bufs=1) as pqkv:
                    qT_sb = pqkv.tile([128, 2, S], BF16)
                    kT_sb = pqkv.tile([128, 2, S], BF16)
                    v_sb = pqkv.tile([128, NST, HPC, 65], BF16)
                    nc.vector.memset(v_sb[:, :, :, 64:65], 1.0)

                    # ---- stage 2: projections qT, kT, v ----
                    with (
                        tc.tile_pool(name="wq", bufs=1) as wq_p,
                        tc.tile_pool(name="xt", bufs=8) as xt_p,
                        tc.tile_pool(name="pj", bufs=2, space="PSUM") as pj_p,
                    ):
                        wq_sb = wq_p.tile([128, NDCH, DC], BF16)
                        wk_sb = wq_p.tile([128, NDCH, DC], BF16)
                        wv_sb = wq_p.tile([128, NDCH, DC], BF16)
                        nc.sync.dma_start(
                            out=wq_sb,
                            in_=wqT.ap().rearrange("(c p) o -> p c o", p=128))
                        nc.sync.dma_start(
                            out=wk_sb,
                            in_=wkT.ap().rearrange("(c p) o -> p c o", p=128))
                        nc.sync.dma_start(
                            out=wv_sb,
                            in_=wvT.ap().rearrange("(c p) o -> p c o", p=128))
                        xt_tiles = []
                        for ch in range(NDCH):
                            xt = xt_p.tile([128, S], BF16, tag="xt",
                                           name=f"xt{ch}")
                            nc.sync.dma_start(
                                out=xt, in_=xT[ch * 128:(ch + 1) * 128, :])
                            xt_tiles.append(xt)
                        for j in range(2):
                            for sc in range(NQC):
                                ps_q = pj_p.tile([128, QC], F32, tag="psq")
                                ps_k = pj_p.tile([128, QC], F32, tag="psk")
                                for ch in range(NDCH):
                                    nc.tensor.matmul(
                                        ps_q,
                                        wq_sb[:, ch, j * 128:(j + 1) * 128],
                                        xt_tiles[ch][:, sc * QC:(sc + 1) * QC],
                                        start=(ch == 0), stop=(ch == NDCH - 1))
                                for ch in range(NDCH):
                                    nc.tensor.matmul(
                                        ps_k,
                                        wk_sb[:, ch, j * 128:(j + 1) * 128],
                                        xt_tiles[ch][:, sc * QC:(sc + 1) * QC],
                                        start=(ch == 0), stop=(ch == NDCH - 1))
                                nc.vector.tensor_scalar(
                                    out=qT_sb[:, j, sc * QC:(sc + 1) * QC],
                                    in0=ps_q, scalar1=bq_sb[:, j:j + 1],
                                    scalar2=SCALE,
                                    op0=mybir.AluOpType.add,
                                    op1=mybir.AluOpType.mult)
                                nc.vector.tensor_scalar(
                                    out=kT_sb[:, j, sc * QC:(sc + 1) * QC],
                                    in0=ps_k, scalar1=bk_sb[:, j:j + 1],
                                    scalar2=None, op0=mybir.AluOpType.add)
                        for st in range(NST):
                            ps_v = pj_p.tile([128, DC], F32, tag="psv")
                            for ch in range(NDCH):
                                nc.tensor.matmul(
                                    ps_v,
                                    xt_tiles[ch][:, st * 128:(st + 1) * 128],
                                    wv_sb[:, ch, :],
                                    start=(ch == 0), stop=(ch == NDCH - 1))
                            nc.vector.tensor_tensor(
                                out=v_sb[:, st, :, 0:64],
                                in0=ps_v.rearrange("p (h d) -> p h d", h=HPC),
                                in1=bv_b.rearrange("p (h d) -> p h d", h=HPC),
                                op=mybir.AluOpType.add)

                    if DEBUG:
                        nc.sync.dma_start(out=dbg["qT"][:], in_=qT_sb)
                        nc.sync.dma_start(out=dbg["kT"][:], in_=kT_sb)
                        nc.sync.dma_start(out=dbg["v"][:], in_=v_sb)

                    # ---- stage 3: attention ----
                    with (
                        tc.tile_pool(name="msk", bufs=NKT + 2) as msk_p,
                        tc.tile_pool(name="sp", bufs=3) as sp_p,
                        tc.tile_pool(name="et", bufs=4) as et_p,
                        tc.tile_pool(name="rc", bufs=4) as rc_p,
                        tc.tile_pool(name="pss", bufs=4, space="PSUM") as pss_p,
                        tc.tile_pool(name="psc", bufs=2, space="PSUM") as psc_p,
                        tc.tile_pool(name="rb", bufs=2, space="PSUM") as rb_p,
                    ):
                        for qc in range(NQC):
                            mtiles = []
                            for kt in range(NKT):
                                mt = msk_p.tile([128, QC], F32, tag="mt",
                                                name=f"mt{qc}_{kt}")
                                nc.gpsimd.indirect_dma_start(
                                    out=mt, out_offset=None, in_=mdist[qc][:],
                                    in_offset=bass.IndirectOffsetOnAxis(
                                        ap=vki[:, kt:kt + 1], axis=0))
                                mtiles.append(mt)
                            for pr in range(2):
                                ctx_ps = [
                                    psc_p.tile([65, QC], F32, tag="ctx",
                                               name=f"ctx_{qc}_{pr}_{i}")
                                    for i in range(2)]
                                for kt in range(NKT):
                                    sp = sp_p.tile([128, 2, QC], F32,
                                                   tag="sp")
                                    for hh in range(2):
                                        lo = hh * 64
                                        ps_s = pss_p.tile([128, QC], F32,
                                                          tag="ps_s")
                                        nc.tensor.matmul(
                                            ps_s,
                                            kT_sb[lo:lo + 64, pr,
                                                  kt * 128:(kt + 1) * 128],
                                            qT_sb[lo:lo + 64, pr,
                                                  qc * QC:(qc + 1) * QC],
                                            start=True, stop=True)
                                        nc.vector.tensor_add(
                                            out=sp[:, hh, :], in0=ps_s,
                                            in1=mtiles[kt])
                                    et = et_p.tile([128, 2, QC], BF16,
                                                   tag="et")
                                    nc.scalar.activation(
                                        out=et, in_=sp,
                                        func=mybir.ActivationFunctionType.Exp)
                                    if DEBUG and qc == 0 and pr == 0 and kt == 0:
                                        nc.sync.dma_start(out=dbg["mt"][:],
                                                          in_=mtiles[0])
                                        nc.sync.dma_start(out=dbg["et"][:],
                                                          in_=et[:, 0, :])
                                    for hh in range(2):
                                        h = 2 * pr + hh
                                        nc.tensor.matmul(
                                            ctx_ps[hh],
                                            v_sb[:, kt, h, :],
                                            et[:, hh, :],
                                            start=(kt == 0),
                                            stop=(kt == NKT - 1))
                                cols = slice(qc * QC, (qc + 1) * QC)
                                for hh in range(2):
                                    # row 64 of ctx_ps = sum_k exp; rows
                                    # 0..63 = unnormalized ctxT [d, q].
                                    # Reciprocal over all 65 rows: only row
                                    # 64 (1/sums) is used, but running all
                                    # lanes beats a 1-lane op by ~5x.
                                    rec = rc_p.tile([65, QC], F32,
                                                    tag="rec")
                                    nc.vector.reciprocal(
                                        out=rec, in_=ctx_ps[hh][0:65, :])
                                    # outer-product broadcast of 1/sum to
                                    # partitions 0..63 via PE
                                    rb = rb_p.tile([64, QC], F32, tag="rb")
                                    nc.tensor.matmul(
                                        rb, ones64[64:65, :],
                                        rec[64:65, :],
                                        start=True, stop=True)
                                    rbs = rc_p.tile([64, QC], F32,
                                                    tag="rbs")
                                    nc.vector.tensor_copy(out=rbs, in_=rb)
                                    if hh == 0:
                                        nc.vector.tensor_tensor(
                                            out=ctxT_sb[0:64, pr, cols],
                                            in0=ctx_ps[hh][0:64, :],
                                            in1=rbs, op=mybir.AluOpType.mult)
                                    else:
                                        tmp = rc_p.tile([64, QC], F32,
                                                        tag="tmp")
                                        nc.vector.tensor_tensor(
                                            out=tmp,
                                            in0=ctx_ps[hh][0:64, :],
                                            in1=rbs, op=mybir.AluOpType.mult)
                                        nc.sync.dma_start(
                                            out=ctxT_sb[64:128, pr, cols],
                                            in_=tmp)

                # ---- stage 4: Wo partial -> part_blk ----
                if DEBUG:
                    nc.sync.dma_start(out=dbg["ctx"][:], in_=ctxT_sb)
                with (
                    tc.tile_pool(name="wo", bufs=1) as wo_p,
                    tc.tile_pool(name="ao", bufs=3) as ao_p,
                    tc.tile_pool(name="pst", bufs=4, space="PSUM") as pst_p,
                ):
                    wo_sb = wo_p.tile([128, 2, D], F32)
                    nc.sync.dma_start(
                        out=wo_sb,
                        in_=woT.ap().rearrange("(c p) o -> p c o", p=128))
                    for ot in range(NDCH):
                        for sc in range(NQC):
                            ps_a = pst_p.tile([128, QC], F32, tag="ps_a")
                            for pr in range(2):
                                nc.tensor.matmul(
                                    ps_a,
                                    wo_sb[:, pr, ot * 128:(ot + 1) * 128],
                                    ctxT_sb[:, pr, sc * QC:(sc + 1) * QC],
                                    start=(pr == 0), stop=(pr == 1))
                            ao = ao_p.tile([128, QC], F32, tag="ao")
                            nc.vector.tensor_scalar(
                                out=ao, in0=ps_a,
                                scalar1=bo4_sb[:, ot:ot + 1], scalar2=None,
                                op0=mybir.AluOpType.add)
                            nc.sync.dma_start(
                                out=part_blk[ot // 2][sc,
                                                      (ot % 2) * 128:
                                                      (ot % 2 + 1) * 128, :],
                                in_=ao)
                        if ot % 2 == 1:
                            i = ot // 2
                            nc.gpsimd.collective_compute(
                                "ReduceScatter", mybir.AluOpType.add,
                                replica_groups=GROUPS,
                                ins=[part_blk[i][:]],
                                outs=[rs_out[i][:]])

            # ---- stage 5: ReduceScatter issued chunked inside stage 4 ----
            if DEBUG:
                for i in range(NDCH // 2):
                    nc.sync.dma_start(
                        out=dbg["part"][i * 256:(i + 1) * 256, :],
                        in_=part_blk[i][0])
                    nc.sync.dma_start(
                        out=dbg["rs"][i * 256:(i + 1) * 256, :],
                        in_=rs_out[i][:])

            # ---- stage 6: Wp + residual + LayerNorm ----
            with (
                tc.tile_pool(name="wp", bufs=1) as wp_p,
                tc.tile_pool(name="rsx", bufs=1) as rsx_p,
                tc.tile_pool(name="o6", bufs=3) as o6_p,
                tc.tile_pool(name="st6", bufs=4) as st6_p,
                tc.tile_pool(name="ps6", bufs=4, space="PSUM") as ps6_p,
            ):
                wp_sb = wp_p.tile([128, NDCH, D], F32)
                nc.sync.dma_start(
                    out=wp_sb, in_=wpT.ap().rearrange("(c p) o -> p c o",
                                                      p=128))
                gamma_b = wp_p.tile([128, D], F32)
                nc.sync.dma_start(out=gamma_b, in_=bcast(gamma[:]))
                beta_b = wp_p.tile([128, D], F32)
                nc.sync.dma_start(out=beta_b, in_=bcast(beta[:]))
                bp_b = wp_p.tile([128, D], F32)
                nc.sync.dma_start(out=bp_b, in_=bcast(bp[:]))
                rsT_sb = rsx_p.tile([128, NDCH, SS], F32)
                for i in range(NDCH // 2):
                    nc.sync.dma_start(
                        out=rsT_sb[:, 2 * i:2 * i + 2, :],
                        in_=rs_out[i].ap().rearrange("(c p) s -> p c s",
                                                     p=128))
                xpb = rsx_p.tile([128, 4, D], F32)
                nc.sync.dma_start(
                    out=xpb, in_=x_sl.ap().rearrange("(t p) o -> p t o",
                                                     p=128))
                for stl in range(4):
                    nc.vector.tensor_add(out=xpb[:, stl, :],
                                         in0=xpb[:, stl, :], in1=bp_b)
                for stl in range(4):
                    o_sb = o6_p.tile([128, D], F32, tag="o")
                    for nh in range(2):
                        ps_o = ps6_p.tile([128, 512], F32, tag="ps_o")
                        for ch in range(NDCH):
                            nc.tensor.matmul(
                                ps_o,
                                rsT_sb[:, ch, stl * 128:(stl + 1) * 128],
                                wp_sb[:, ch, nh * 512:(nh + 1) * 512],
                                start=(ch == 0), stop=(ch == NDCH - 1))
                        nc.vector.tensor_tensor(
                            out=o_sb[:, nh * 512:(nh + 1) * 512], in0=ps_o,
                            in1=xpb[:, stl, nh * 512:(nh + 1) * 512],
                            op=mybir.AluOpType.add)
                    if DEBUG and stl == 0:
                        nc.sync.dma_start(out=dbg["opre"][:], in_=o_sb)
                    stats = st6_p.tile([128, 2, 6], F32, tag="stats")
                    for i in range(2):
                        nc.vector.bn_stats(
                            out=stats[:, i, :],
                            in_=o_sb[:, i * 512:(i + 1) * 512])
                    mv = st6_p.tile([128, 2], F32, tag="mv")
                    nc.vector.bn_aggr(out=mv, in_=stats)
                    sd = st6_p.tile([128, 1], F32, tag="sd")
                    nc.scalar.activation(
                        out=sd, in_=mv[:, 1:2],
                        func=mybir.ActivationFunctionType.Sqrt,
                        bias=eps_t, scale=1.0)
                    rstd = st6_p.tile([128, 1], F32, tag="rstd")
                    nc.vector.reciprocal(out=rstd, in_=sd)
                    nc.vector.tensor_scalar(
                        out=o_sb, in0=o_sb, scalar1=mv[:, 0:1], scalar2=rstd,
                        op0=mybir.AluOpType.subtract, op1=mybir.AluOpType.mult)
                    nc.vector.tensor_mul(out=o_sb, in0=o_sb, in1=gamma_b)
                    nc.vector.tensor_add(out=o_sb, in0=o_sb, in1=beta_b)
                    nc.sync.dma_start(
                        out=out[stl * 128:(stl + 1) * 128, :], in_=o_sb)

    nc.compile()
    return nc


def _prep_inputs(x, verse_positions, W_q, b_q, W_k, b_k, W_v, b_v,
                 W_o, b_o, W_p, b_p, gamma, beta):
    import ml_dtypes
    f = np.float32
    bf = ml_dtypes.bfloat16
    x = np.asarray(x, f)
    vp = np.asarray(verse_positions)
    in_maps = []
    wpT = np.ascontiguousarray(np.asarray(W_p, f).T)
    xTb = [np.ascontiguousarray(x[b].T).astype(bf) for b in range(B)]
    for c in range(8):
        b, r = divmod(c, 4)
        sl = slice(DC * r, DC * (r + 1))
        vpb = vp[b].astype(np.int32)
        in_maps.append({
            "xT": xTb[b],
            "x_sl": np.ascontiguousarray(x[b, SS * r:SS * (r + 1), :]),
            "wqT": np.ascontiguousarray(np.asarray(W_q, f)[sl, :].T).astype(bf),
            "wkT": np.ascontiguousarray(np.asarray(W_k, f)[sl, :].T).astype(bf),
            "wvT": np.ascontiguousarray(np.asarray(W_v, f)[sl, :].T).astype(bf),
            "woT": np.ascontiguousarray(np.asarray(W_o, f)[:, sl].T),
            "wpT": wpT,
            "bq": np.asarray(b_q, f)[sl].reshape(2, 128).T.copy(),
            "bk": np.asarray(b_k, f)[sl].reshape(2, 128).T.copy(),
            "bv": np.asarray(b_v, f)[sl].copy(),
            "bo4": (np.asarray(b_o, f) / 4.0).reshape(NDCH, 128).T.copy(),
            "bp": np.asarray(b_p, f).copy(),
            "gamma": np.asarray(gamma, f).copy(),
            "beta": np.asarray(beta, f).copy(),
            "vq": vpb.astype(f),
            "vk_idx": vpb.reshape(NKT, 128).T.copy(),
        })
    return in_maps


def kernel(**inputs):
    if "nc" not in _CACHE:
        _CACHE["nc"] = _build()
    nc = _CACHE["nc"]
    in_maps = _prep_inputs(**inputs)
    res = run_bass_kernel_spmd(nc, in_maps, core_ids=list(range(8)))
    _CACHE["last_res"] = res
    out = np.empty((B, S, D), np.float32)
    for c in range(8):
        b, r = divmod(c, 4)
        out[b, SS * r:SS * (r + 1), :] = res.results[c]["out"]
    return out
